# revision 52
# baseline (speedup 1.0000x reference)
"""Bayesian uncertainty distance kernel for TRN2 (8 NeuronCores, SPMD).

Math (per reference):
    W_s  = weight_mu + eps_w[s] * softplus(weight_rho)          [S,D,D]
    b_s  = bias_mu   + eps_b[s] * softplus(bias_rho)            [S,D]
    qt_s = query @ W_s + b_s                                    [S,Q,D]
    d2_s = ||qt_s||^2 - 2 qt_s.proto^T + ||proto||^2            [S,Q,P]
    mean = mean_s sqrt(d2_s);  std = std_s(sqrt(d2_s), ddof=1)

Sharding: data-parallel over Q (8192 -> 8 x 1024). Everything else replicated.

On-chip design (per core, Q=1024, P=2048, D=256, S=10), ~405us measured:
  - samples are DEFINED as x_s := fp16(-2*(query@W_s + b_s)) so that every
    moment is computed consistently from the same rounded values; first-order
    fp16 rounding error then cancels exactly in the variance (an inconsistent
    16-bit path measured 27% std error from catastrophic cancellation in
    E[d^2]-E[d]^2; this consistent one measures ~1.6e-3).
  - phase 1 per s: fp16 qt matmuls (W_s stationary, query^T moving) ->
    x_s = DVE tensor_scalar(psum*-2 + (-2 b_s)) -> fp16 [e,q] SBUF;
    x2 = ACT Square(x_s); qn rows = ones-stationary matmul of x2
    (scale 0.25 on the psum->sbuf copy), stored as fp16 rows with a
    companion all-ones row for the rank-2 update below.
  - xsum = sum_s x_s via identity-matmul PSUM accumulation (mixed-dtype
    DVE tensor_tensor measured 13x slow); qnsum row = DVE reduce of qn rows.
  - phase 2 per (qtile, s): PSUM d2 = rank-2([qn_s;1] x [1;pn]) +
    x_s-block @ proto^T (fp16, K=2x128, lhsT-major order to minimize
    LDWEIGHTS boundaries); dist = ACT Sqrt(psum), no bias needed;
    macc += dist (DVE fp32).
  - variance via sum-of-d2: ss = rank-2([qnsum;1] x [1;10*pn]) +
    xsum @ proto^T in fp32; u = ss - macc^2/10 (DVE); std = Sqrt(u/9).
  - mean = macc/10 on DVE (gpsimd tensor_scalar measured 29us/tile).

The host does only O(S*D^2) prep in numpy (softplus, W_s, transposes, pn).
"""

import os
import numpy as np

import concourse.bass as bass
import concourse.mybir as mybir
import concourse.tile as tile
from concourse import bacc, bass_utils

AF = mybir.ActivationFunctionType
ALU = mybir.AluOpType

# Note: walrus's --enable-ldw-opt stays false — fp32 matmuls emit
# InstLdweights that are "not compatible with LDW optimization".
F32 = mybir.dt.float32
F16 = mybir.dt.float16
F8 = mybir.dt.float8e4
DR = mybir.MatmulPerfMode.DoubleRow

import ml_dtypes
NP_F8 = ml_dtypes.float8_e4m3  # TRN float8e4: max normal +-240

NCORES = 8
D = 256
Q_FULL = 8192
P = 2048
S = 10
QLOC = Q_FULL // NCORES  # 1024
ET = D // 128  # 2 e-tiles
DT = D // 128  # 2 d-tiles
QT = QLOC // 128  # 8 q-tiles per core
PC = P // 512  # 4 p-chunks
QC = QLOC // 512  # 2 q-chunks

_CACHE = {}
LAST_RESULTS = None


def _build_bass():
    nc = bacc.Bacc(
        "TRN2",
        target_bir_lowering=False,
        debug=False,
        num_devices=NCORES,
    )
    ins = {}
    ins["qT16"] = nc.dram_tensor("qT16", [128, DT * QLOC], F16, kind="ExternalInput").ap()
    ins["W16"] = nc.dram_tensor("W16", [S, 128, DT * 256], F16, kind="ExternalInput").ap()
    ins["b2T"] = nc.dram_tensor("b2T", [128, ET * S], F32, kind="ExternalInput").ap()
    ins["yT16"] = nc.dram_tensor("yT16", [128, ET * P], F16, kind="ExternalInput").ap()
    ins["y8"] = nc.dram_tensor("y8", [128, ET, P], F8, kind="ExternalInput").ap()
    ins["yext16"] = nc.dram_tensor("yext16", [2, P], F16, kind="ExternalInput").ap()
    ins["pn10_16"] = nc.dram_tensor("pn10_16", [1, P], F16, kind="ExternalInput").ap()
    ins["onesr16"] = nc.dram_tensor("onesr16", [1, 128], F16, kind="ExternalInput").ap()
    ins["eyeT1"] = nc.dram_tensor("eyeT1", [1, 1], F32, kind="ExternalInput").ap()
    ins["o16c"] = nc.dram_tensor("o16c", [128, 1], F16, kind="ExternalInput").ap()
    ins["eye16"] = nc.dram_tensor("eye16", [128, 128], F16, kind="ExternalInput").ap()
    mean_o = nc.dram_tensor("mean_o", [QLOC, P], F32, kind="ExternalOutput").ap()
    std_o = nc.dram_tensor("std_o", [QLOC, P], F32, kind="ExternalOutput").ap()

    with tile.TileContext(nc) as tc:
        _kernel_body(tc, ins, mean_o, std_o)
    nc.compile()
    return nc


def _kernel_body(tc, ins, mean_o, std_o):
    nc = tc.nc
    from contextlib import ExitStack

    ctx = ExitStack()
    with ctx:
        cpool = ctx.enter_context(tc.tile_pool(name="consts", bufs=1))
        wpool = ctx.enter_context(tc.tile_pool(name="wpool", bufs=2))
        xpool = ctx.enter_context(tc.tile_pool(name="xpool", bufs=3))
        xhipool = ctx.enter_context(tc.tile_pool(name="xhipool", bufs=S))
        xlopool = ctx.enter_context(tc.tile_pool(name="xlopool", bufs=S))
        h16pool = ctx.enter_context(tc.tile_pool(name="h16pool", bufs=2))
        x2pool = ctx.enter_context(tc.tile_pool(name="x2pool", bufs=2))
        xsumpool = ctx.enter_context(tc.tile_pool(name="xsumpool", bufs=1))
        qnpool = ctx.enter_context(tc.tile_pool(name="qnpool", bufs=1))
        distpool = ctx.enter_context(tc.tile_pool(name="distpool", bufs=3))
        maccpool = ctx.enter_context(tc.tile_pool(name="maccpool", bufs=2))
        finpool = ctx.enter_context(tc.tile_pool(name="finpool", bufs=2))
        outpool = ctx.enter_context(tc.tile_pool(name="outpool", bufs=3))
        pp = ctx.enter_context(tc.tile_pool(name="pp", bufs=4, space="PSUM"))

        # ---- constants into SBUF ----
        qT_t = cpool.tile([128, DT * QLOC], F16)
        nc.sync.dma_start(qT_t[:], ins["qT16"])
        b2_t = cpool.tile([128, ET * S], F32)
        nc.sync.dma_start(b2_t[:], ins["b2T"])
        yT16_t = cpool.tile([128, ET * P], F16)
        nc.sync.dma_start(yT16_t[:], ins["yT16"])
        y8_t = cpool.tile([128, ET, P], F8)
        nc.sync.dma_start(y8_t[:], ins["y8"])
        yext16_t = cpool.tile([2, P], F16)
        nc.sync.dma_start(yext16_t[:], ins["yext16"])
        pn10_t = cpool.tile([1, P], F16)
        nc.sync.dma_start(pn10_t[:], ins["pn10_16"])
        onesr16_t = cpool.tile([1, 128], F16)
        nc.sync.dma_start(onesr16_t[:], ins["onesr16"])
        eyeT1_t = cpool.tile([1, 1], F32)
        nc.sync.dma_start(eyeT1_t[:], ins["eyeT1"])
        o16c_t = cpool.tile([128, 1], F16)
        nc.sync.dma_start(o16c_t[:], ins["o16c"])
        eye16_t = cpool.tile([128, 128], F16)
        nc.sync.dma_start(eye16_t[:], ins["eye16"])

        xsum_t = xsumpool.tile([128, ET * QLOC], F16)
        qn9 = qnpool.tile([128, QT], F32)  # qnsum/(S-1) bias columns for std
        # qn rows (fp16, max qn ~55k < 65504): row 0 holds qn for all (s,q),
        # row 1 is ones; [2,128] slices feed the rank-2 (qn+pn) matmul.
        qrow16_t = qnpool.tile([2, S * QLOC], F16)
        nc.vector.memset(qrow16_t[0:2, :], 1.0)
        # ss-side rank-2 operand: row 0 = qnsum (fp32), row 1 = ones
        qsrow32_t = qnpool.tile([2, QLOC], F32)
        nc.vector.memset(qsrow32_t[0:2, :], 1.0)

        x_tiles = []
        x8_tiles = []
        # ---------- phase 1: per-sample transformed queries ----------
        for s in range(S):
            w_t = wpool.tile([128, DT * 256], F16, tag="w")
            nc.sync.dma_start(w_t[:], ins["W16"][s])
            x_t = xpool.tile([128, ET * QLOC], F16, tag="x", name=f"x{s}")
            x_tiles.append(x_t)
            x2s = []
            for et in range(ET):
                for qc in range(QC):
                    qp = pp.tile([128, 512], F32, tag="ps", name=f"qp{s}_{et}_{qc}")
                    for dt_ in range(DT):
                        nc.tensor.matmul(
                            qp[:],
                            lhsT=w_t[:, dt_ * 256 + et * 128 : dt_ * 256 + et * 128 + 128],
                            rhs=qT_t[:, dt_ * QLOC + qc * 512 : dt_ * QLOC + qc * 512 + 512],
                            start=(dt_ == 0),
                            stop=(dt_ == DT - 1),
                        )
                    # x = fp16(-2*qt - 2*b) on DVE: (psum * -2) + b2col
                    # (keeps phase-1 ACT light so the PE stream stays dense)
                    nc.vector.tensor_scalar(
                        x_t[:, et * QLOC + qc * 512 : et * QLOC + qc * 512 + 512],
                        qp[:],
                        -2.0,
                        b2_t[:, et * S + s : et * S + s + 1],
                        ALU.mult,
                        ALU.add,
                    )
                x2_t = x2pool.tile([128, QLOC], F16, tag=f"x2_{et}", name=f"x2_{s}_{et}")
                x2s.append(x2_t)
                # x2 = x^2 = 4*qt^2 on ACT (phase 1 is DVE-bound; the 0.25
                # compensation is folded into the qn psum->sbuf copy scale)
                nc.scalar.square(x2_t[:], x_t[:, et * QLOC : (et + 1) * QLOC])
            # two-term fp8 split x ~= hi + lo (residual ~2^-9 relative) so
            # phase 2 can run the cross as DoubleRow fp8 (0.5 cyc/col)
            xhi_t = xhipool.tile([128, ET, QLOC], F8, tag="xh", name=f"xh{s}")
            nc.vector.tensor_copy(xhi_t[:].rearrange("p a b -> p (a b)"), x_t[:])
            h16_t = h16pool.tile([128, ET * QLOC], F16, tag="h16", name=f"h16_{s}")
            nc.scalar.copy(h16_t[:], xhi_t[:].rearrange("p a b -> p (a b)"))
            xlo_t = xlopool.tile([128, ET, QLOC], F8, tag="xl", name=f"xl{s}")
            nc.vector.tensor_tensor(
                xlo_t[:].rearrange("p a b -> p (a b)"), x_t[:], h16_t[:], ALU.subtract
            )
            x8_tiles.append((xhi_t, xlo_t))
            # xsum += x_s on DVE (fp16 2x mode; the accumulation drift is
            # ~0.3/elem worst case -> ~3 absolute on the d2 sum, negligible)
            if s == 0:
                nc.vector.tensor_copy(xsum_t[:], x_t[:])
            else:
                nc.vector.tensor_add(xsum_t[:], xsum_t[:], x_t[:])
            # qn rows: ones-stationary matmuls (shared lhsT, no LDW tax);
            # 0.25 compensates x2 = (2*qt)^2
            for qc in range(QC):
                qr_p = pp.tile([1, 512], F32, tag="ps", name=f"qr{s}_{qc}")
                for et in range(ET):
                    nc.tensor.matmul(
                        qr_p[:],
                        lhsT=o16c_t[:],
                        rhs=x2s[et][:, qc * 512 : (qc + 1) * 512],
                        start=(et == 0),
                        stop=(et == ET - 1),
                    )
                nc.scalar.mul(
                    qrow16_t[0:1, s * QLOC + qc * 512 : s * QLOC + qc * 512 + 512],
                    qr_p[:],
                    0.25,
                )



        # qnsum row (fp32) = sum_s of the fp16 qn rows, consistent with the
        # per-sample values the rank-2 matmuls use
        nc.vector.tensor_reduce(
            qsrow32_t[0:1, :],
            qrow16_t[0:1, :].rearrange("p (s q) -> p q s", s=S),
            axis=mybir.AxisListType.X,
            op=ALU.add,
        )
        # qnsum/(S-1) as per-partition bias columns (PE transpose of the
        # qnsum row) so the ss matmuls can drop the fp32 rank-2 entirely
        for qt_ in range(QT):
            qsp = pp.tile([128, 1], F32, tag="ps", name=f"qsp{qt_}")
            nc.tensor.matmul(
                qsp[:],
                lhsT=qsrow32_t[0:1, qt_ * 128 : qt_ * 128 + 128],
                rhs=eyeT1_t[:],
                is_transpose=True,
            )
            nc.scalar.mul(qn9[:, qt_ : qt_ + 1], qsp[:], 1.0 / (S - 1))

        # ---------- phase 2: distances, moments, outputs ----------
        PH = 1024  # psum tile width (2 banks); 4 bufs deepen the PE pipeline
        NH = P // PH
        for qt_ in range(QT):
            macc_t = maccpool.tile([128, P], F32, tag="macc", name=f"macc{qt_}")
            for s in range(S):
                dist_t = None
                if s > 0:
                    dist_t = distpool.tile([128, P], F32, tag="dist", name=f"d{qt_}_{s}")
                cps = [
                    pp.tile([128, PH], F32, tag="ps", name=f"cp{qt_}_{s}_{h}")
                    for h in range(NH)
                ]
                # lhsT-major ordering: each stationary operand covers all
                # PSUM halves before switching (leader-MM LDW tax once per
                # lhsT instead of once per half)
                lhsT_r2 = qrow16_t[:, s * QLOC + qt_ * 128 : s * QLOC + qt_ * 128 + 128]
                for h in range(NH):
                    for pc in range(PH // 512):
                        o = h * PH + pc * 512
                        nc.tensor.matmul(
                            cps[h][:, pc * 512 : (pc + 1) * 512],
                            lhsT=lhsT_r2,
                            rhs=yext16_t[:, o : o + 512],
                            start=True,
                            stop=False,
                            skip_group_check=True,
                        )
                xhi_t, xlo_t = x8_tiles[s]
                for half in (xhi_t, xlo_t):
                    lhs = half[:, :, qt_ * 128 : qt_ * 128 + 128]
                    for h in range(NH):
                        for pc in range(PH // 512):
                            o = h * PH + pc * 512
                            # DoubleRow fp8: K=256 in one instruction
                            nc.tensor.matmul(
                                cps[h][:, pc * 512 : (pc + 1) * 512],
                                lhsT=lhs,
                                rhs=y8_t[:, :, o : o + 512],
                                start=False,
                                stop=(half is xlo_t),
                                perf_mode=DR,
                                skip_group_check=True,
                            )
                dst = macc_t if s == 0 else dist_t
                for h in range(NH):
                    nc.scalar.activation(
                        dst[:, h * PH : (h + 1) * PH], cps[h][:], AF.Sqrt
                    )
                if s > 0:
                    nc.vector.tensor_add(macc_t[:], macc_t[:], dist_t[:])

            # sum_s d2 = qnsum + 10*pn + xsum.proto^T (fp32, rank-2 + cross)
            # m2 = macc^2; u = ssp - m2/10  (all on DVE, ACT stays on sqrt)
            m2_t = finpool.tile([128, P], F32, tag="fin", name=f"m2{qt_}")
            nc.vector.tensor_mul(m2_t[:], macc_t[:], macc_t[:])
            u_t = finpool.tile([128, P], F32, tag="fin", name=f"u{qt_}")
            ssps = [
                pp.tile([128, PH], F32, tag="ps", name=f"ssp{qt_}_{h}")
                for h in range(NH)
            ]
            # rank-1 10*pn seed + fp16 cross (exact accumulation); qnsum
            # joins at the final Sqrt as a per-partition bias
            for h in range(NH):
                for pc in range(PH // 512):
                    o = h * PH + pc * 512
                    nc.tensor.matmul(
                        ssps[h][:, pc * 512 : (pc + 1) * 512],
                        lhsT=onesr16_t[:],
                        rhs=pn10_t[:, o : o + 512],
                        start=True,
                        stop=False,
                        skip_group_check=True,
                    )
            for et in range(ET):
                lhs = xsum_t[:, et * QLOC + qt_ * 128 : et * QLOC + qt_ * 128 + 128]
                for h in range(NH):
                    for pc in range(PH // 512):
                        o = h * PH + pc * 512
                        nc.tensor.matmul(
                            ssps[h][:, pc * 512 : (pc + 1) * 512],
                            lhsT=lhs,
                            rhs=yT16_t[:, et * P + o : et * P + o + 512],
                            start=False,
                            stop=(et == ET - 1),
                            skip_group_check=True,
                        )
            for h in range(NH):
                nc.vector.scalar_tensor_tensor(
                    u_t[:, h * PH : (h + 1) * PH],
                    m2_t[:, h * PH : (h + 1) * PH],
                    -1.0 / S,
                    ssps[h][:],
                    ALU.mult,
                    ALU.add,
                )
            ostd_t = outpool.tile([128, P], F32, tag="out", name=f"os{qt_}")
            nc.scalar.activation(
                ostd_t[:], u_t[:], AF.Sqrt,
                bias=qn9[:, qt_ : qt_ + 1],
                scale=1.0 / (S - 1),
            )
            omean_t = outpool.tile([128, P], F32, tag="out", name=f"om{qt_}")
            nc.vector.tensor_scalar_mul(omean_t[:], macc_t[:], 1.0 / S)
            nc.sync.dma_start(std_o[qt_ * 128 : (qt_ + 1) * 128, :], ostd_t[:])
            nc.sync.dma_start(mean_o[qt_ * 128 : (qt_ + 1) * 128, :], omean_t[:])


def _prep_inputs(query_features, prototypes, weight_mu, weight_rho, bias_mu, bias_rho, eps_w, eps_b):
    f32, f16 = np.float32, np.float16
    sp_w = np.log1p(np.exp(weight_rho.astype(np.float64))).astype(f32)
    sp_b = np.log1p(np.exp(bias_rho.astype(np.float64))).astype(f32)
    W = (weight_mu[None] + eps_w * sp_w[None]).astype(f32)  # [S,D,D]
    B = (bias_mu[None] + eps_b * sp_b[None]).astype(f32)  # [S,D]
    Wh = W.astype(f16)
    qfh = query_features.astype(f16)  # [Q,D]
    # prototypes quantized once to fp8; yT16/pn derive from y8 so the fp16
    # ss cross and the fp8 DoubleRow cross see the same prototype values
    y8 = prototypes.astype(f32).astype(NP_F8)  # [P,D]
    yh = y8.astype(f16)  # exact upconvert
    pn = (y8.astype(f32) ** 2).sum(-1, dtype=f32)  # [P]
    pn16 = pn.astype(f16)
    pn10 = (float(S) * pn16.astype(f32)).astype(f32)
    b2 = (-2.0 * B).astype(f32)  # [S,D]

    W16 = np.ascontiguousarray(
        Wh.reshape(S, DT, 128, 256).transpose(0, 2, 1, 3).reshape(S, 128, DT * 256)
    )
    b2T = np.ascontiguousarray(
        b2.T.reshape(ET, 128, S).transpose(1, 0, 2).reshape(128, ET * S)
    )
    yT16 = np.ascontiguousarray(
        yh.T.reshape(ET, 128, P).transpose(1, 0, 2).reshape(128, ET * P)
    )
    y8T = np.ascontiguousarray(y8.T.reshape(ET, 128, P).transpose(1, 0, 2))
    yext16 = np.stack([np.ones(P, f16), pn16]).astype(f16)  # [2,P]
    pn10_16 = pn10.astype(f16)[None, :]  # [1,P]
    common = {
        "W16": W16,
        "b2T": b2T,
        "yT16": yT16,
        "y8": y8T,
        "yext16": yext16,
        "pn10_16": pn10_16,
        "onesr16": np.ones((1, 128), f16),
        "eyeT1": np.eye(1, dtype=f32),
        "o16c": np.ones((128, 1), f16),
        "eye16": np.eye(128, dtype=f16),
    }
    in_maps = []
    for c in range(NCORES):
        qs = qfh[c * QLOC : (c + 1) * QLOC]  # [QLOC, D]
        qT16 = np.ascontiguousarray(
            qs.T.reshape(DT, 128, QLOC).transpose(1, 0, 2).reshape(128, DT * QLOC)
        )
        in_maps.append({"qT16": qT16, **common})
    return in_maps


def kernel(**inputs):
    global LAST_RESULTS
    n_samples = int(inputs.pop("n_samples", S))
    assert n_samples == S, f"kernel hardcodes S={S}, got {n_samples}"
    np_inputs = {
        k: np.asarray(v, dtype=np.float32)
        for k, v in inputs.items()
    }
    in_maps = _prep_inputs(**np_inputs)

    if "nc" not in _CACHE:
        _CACHE["nc"] = _build_bass()
    nc = _CACHE["nc"]

    trace = bool(int(os.environ.get("KERNEL_TRACE", "0")))
    res = bass_utils.run_bass_kernel_spmd(
        nc, in_maps, core_ids=list(range(NCORES)), trace=trace
    )
    LAST_RESULTS = res
    mean = np.concatenate([r["mean_o"] for r in res.results], axis=0)
    std = np.concatenate([r["std_o"] for r in res.results], axis=0)
    return mean, std



# revision 62
# speedup vs baseline: 1.1238x; 1.1238x over previous
"""Bayesian uncertainty distance kernel for TRN2 (8 NeuronCores, SPMD).

Math (per reference):
    W_s  = weight_mu + eps_w[s] * softplus(weight_rho)          [S,D,D]
    b_s  = bias_mu   + eps_b[s] * softplus(bias_rho)            [S,D]
    qt_s = query @ W_s + b_s                                    [S,Q,D]
    d2_s = ||qt_s||^2 - 2 qt_s.proto^T + ||proto||^2            [S,Q,P]
    mean = mean_s sqrt(d2_s);  std = std_s(sqrt(d2_s), ddof=1)

Sharding: data-parallel over Q (8192 -> 8 x 1024). Everything else replicated.

v3 design (per core, Q=1024, P=2048, D=256, S=10):
  - samples are DEFINED as x8_s := fp8e4(-2*(query@W_s + b_s)); all moments
    are derived consistently from these fp8 values so rounding cancels in
    the variance to first order.  prototypes y8 := fp8e4(proto); the pn
    norm and the fp16 copy yT16 are derived FROM y8 for exact consistency.
  - phase 1 per s: fp16 qt matmuls -> DVE tensor_scalar writes x8 directly
    (psum*-2 + -2b, fp8 out); x16u = up(x8) exact; x2 = x16u^2 (DVE fp16);
    qn columns via width-1 PE matmuls (lhsT=x2 128-col slices, rhs=ones col)
    accumulated into one [128, 80] psum tile; xsum psum += eye8 @ x8_s.
  - phase 2 per (qtile, phalf): per s a [128,1024] psum tile gets
    pn via a DoubleRow fp8 seed (ones8x [1,2,128] x pn8x [1,2,P], value
    2*fp8(pn/2)) + the cross term via DoubleRow fp8 matmuls (K=256 in one
    instr, 0.5 cyc/col); dist_s = ACT Sqrt(psum + qn_s column bias) ->
    fp32 SBUF; the s-sum runs on the PE as identity-matmul accumulation
    with float32r (1 cyc/col, fp22 truncation compensated by COMP scale)
    instead of DVE fp32 adds (1x-mode, was the 165us DVE bottleneck).
  - variance via sum-of-d2: ss psum = ones x 10pn16 + xsum16 @ yT16 (fp16);
    u = ss - macc^2*COMP^2/10 (DVE stt); std = Sqrt(u/(S-1) + qnsum/(S-1))
    with the qnsum column as the ACT bias; mean = macc*COMP/10 (DVE ts).

The host does only O(S*D^2) prep in numpy (softplus, W_s, transposes, pn).
"""

import os
import numpy as np
import ml_dtypes

import concourse.bass as bass
import concourse.mybir as mybir
import concourse.tile as tile
from concourse import bacc, bass_utils

AF = mybir.ActivationFunctionType
ALU = mybir.AluOpType
DR = mybir.MatmulPerfMode.DoubleRow

F32 = mybir.dt.float32
F32R = mybir.dt.float32r
F16 = mybir.dt.float16
F8 = mybir.dt.float8e4
NP_F8 = ml_dtypes.float8_e4m3  # TRN float8e4: max normal +-240, has inf

NCORES = 8
D = 256
Q_FULL = 8192
P = 2048
S = 10
QLOC = Q_FULL // NCORES  # 1024
ET = D // 128  # 2 e-tiles
DT = D // 128  # 2 d-tiles
QT = QLOC // 128  # 8 q-tiles per core
PH = 2048  # phase-2 psum tile width (4 banks)
NPH = P // PH  # 1

# fp22 truncation compensation: the PE reads float32r by truncating the
# mantissa to 13 bits, losing an average of 2^-14 relative on the positive
# dist values; COMP re-centres macc (validated against the reference).
COMP = 1.0

_CACHE = {}
LAST_RESULTS = None


def _build_bass():
    nc = bacc.Bacc(
        "TRN2",
        target_bir_lowering=False,
        debug=False,
        num_devices=NCORES,
    )
    ins = {}
    ins["qT16"] = nc.dram_tensor("qT16", [128, DT * QLOC], F16, kind="ExternalInput").ap()
    ins["W16"] = nc.dram_tensor("W16", [S, 128, DT * 256], F16, kind="ExternalInput").ap()
    ins["b2T"] = nc.dram_tensor("b2T", [128, ET * S], F32, kind="ExternalInput").ap()
    ins["yT16"] = nc.dram_tensor("yT16", [128, ET, P], F16, kind="ExternalInput").ap()
    ins["pn16q"] = nc.dram_tensor("pn16q", [1, P], F16, kind="ExternalInput").ap()
    ins["pn10_16"] = nc.dram_tensor("pn10_16", [1, P], F16, kind="ExternalInput").ap()
    ins["onesr16"] = nc.dram_tensor("onesr16", [1, 128], F16, kind="ExternalInput").ap()
    ins["o16c"] = nc.dram_tensor("o16c", [128, 1], F16, kind="ExternalInput").ap()
    ins["eye16"] = nc.dram_tensor("eye16", [128, 128], F16, kind="ExternalInput").ap()
    mean_o = nc.dram_tensor("mean_o", [QLOC, P], F32, kind="ExternalOutput").ap()
    std_o = nc.dram_tensor("std_o", [QLOC, P], F32, kind="ExternalOutput").ap()

    with tile.TileContext(nc) as tc:
        _kernel_body(tc, ins, mean_o, std_o)
    nc.compile()
    return nc


def _kernel_body(tc, ins, mean_o, std_o):
    nc = tc.nc
    from contextlib import ExitStack

    ctx = ExitStack()
    with ctx:
        cpool = ctx.enter_context(tc.tile_pool(name="consts", bufs=1))
        wpool = ctx.enter_context(tc.tile_pool(name="wpool", bufs=2))
        dpool = ctx.enter_context(tc.tile_pool(name="dpool", bufs=S))
        x16pool = ctx.enter_context(tc.tile_pool(name="x16p", bufs=3))
        x2pool = ctx.enter_context(tc.tile_pool(name="x2pool", bufs=2))
        xsumpool = ctx.enter_context(tc.tile_pool(name="xsumpool", bufs=1))
        qnpool = ctx.enter_context(tc.tile_pool(name="qnpool", bufs=1))
        distpool = ctx.enter_context(tc.tile_pool(name="distpool", bufs=2))
        finpool = ctx.enter_context(tc.tile_pool(name="finpool", bufs=2))
        outpool = ctx.enter_context(tc.tile_pool(name="outpool", bufs=3))

        # ---- constants into SBUF ----
        qT_t = cpool.tile([128, DT * QLOC], F16)
        nc.sync.dma_start(qT_t[:], ins["qT16"])
        b2_t = cpool.tile([128, ET * S], F32)
        nc.sync.dma_start(b2_t[:], ins["b2T"])
        yT16_t = cpool.tile([128, ET, P], F16)
        nc.sync.dma_start(yT16_t[:], ins["yT16"])
        pn16q_t = cpool.tile([1, P], F16)
        nc.sync.dma_start(pn16q_t[:], ins["pn16q"])
        pn10_t = cpool.tile([1, P], F16)
        nc.sync.dma_start(pn10_t[:], ins["pn10_16"])
        onesr16_t = cpool.tile([1, 128], F16)
        nc.sync.dma_start(onesr16_t[:], ins["onesr16"])
        o16c_t = cpool.tile([128, 1], F16)
        nc.sync.dma_start(o16c_t[:], ins["o16c"])
        eye16_t = cpool.tile([128, 128], F16)
        nc.sync.dma_start(eye16_t[:], ins["eye16"])

        xsum16_t = xsumpool.tile([128, ET, QLOC], F16)
        # qn columns: [128, QT, S] fp32; [128,1] slices feed the ACT Sqrt bias
        qncol_t = qnpool.tile([128, QT, S], F32)
        qn9r_t = qnpool.tile([128, QT], F32)
        qn9_t = qnpool.tile([128, QT], F32)  # qnsum/(S-1) bias columns for std

        x_tiles = []
        # ---------- phase 1: per-sample fp8 transformed queries ----------
        with tc.tile_pool(name="pp1", bufs=2, space="PSUM") as pp1, \
             tc.tile_pool(name="ppqn", bufs=1, space="PSUM") as ppqn, \
             tc.tile_pool(name="ppxs", bufs=1, space="PSUM") as ppxs:
            qncolp = ppqn.tile([128, QT * S], F32)
            xsump = ppxs.tile([128, ET * QLOC], F32)
            x16_list = []
            for s in range(S):
                w_t = wpool.tile([128, DT * 256], F16, tag="w")
                nc.sync.dma_start(w_t[:], ins["W16"][s])
                # s<2 tiles start the two chains and must survive into
                # phase 2, so they come from the persistent delta pool
                if s < 2:
                    x16_s = dpool.tile([128, ET, QLOC], F16, tag="d16", name=f"x16_{s}")
                else:
                    x16_s = x16pool.tile([128, ET, QLOC], F16, tag="x16", name=f"x16_{s}")
                x16_list.append(x16_s)
                for et in range(ET):
                    for qc in range(2):
                        qp = pp1.tile([128, 512], F32, tag="ps", name=f"qp{s}_{et}_{qc}")
                        for dt_ in range(DT):
                            nc.tensor.matmul(
                                qp[:],
                                lhsT=w_t[:, dt_ * 256 + et * 128 : dt_ * 256 + et * 128 + 128],
                                rhs=qT_t[:, dt_ * QLOC + qc * 512 : dt_ * QLOC + qc * 512 + 512],
                                start=(dt_ == 0),
                                stop=(dt_ == DT - 1),
                            )
                        # x16 = fp16(-2*qt - 2*b) from psum on DVE
                        nc.vector.tensor_scalar(
                            x16_s[:, et, qc * 512 : qc * 512 + 512],
                            qp[:],
                            -2.0,
                            b2_t[:, et * S + s : et * S + s + 1],
                            ALU.mult,
                            ALU.add,
                        )
                # delta chains over stride-2 samples: the phase-2 psum keeps
                # pn + x_path.y alive across the chain, so only the delta is
                # multiplied each step (no per-sample rank-2 reseeding, which
                # was ~25% of all PE matmul issue slots).  fp16 deltas round
                # at ~2^-11|delta| per step, small enough for the std (fp8
                # deltas measured 2.3e-2 std error; fp16 keeps it at ~4e-3).
                if s < 2:
                    d16_s = x16_s
                else:
                    d16_s = dpool.tile([128, ET, QLOC], F16, tag="d16", name=f"dd{s}")
                    nc.vector.tensor_tensor(
                        d16_s[:], x16_s[:], x16_list[s - 2][:], ALU.subtract
                    )
                x_tiles.append(d16_s)
                x2_s = x2pool.tile([128, ET, QLOC], F16, tag="x2", name=f"x2_{s}")
                nc.scalar.square(x2_s[:], x16_s[:])
                # qn columns: width-1 matmuls, one column per (qtile, s)
                for qt8 in range(QT):
                    for et in range(ET):
                        nc.tensor.matmul(
                            qncolp[:, qt8 * S + s : qt8 * S + s + 1],
                            lhsT=x2_s[:, et, qt8 * 128 : qt8 * 128 + 128],
                            rhs=o16c_t[:],
                            start=(et == 0),
                            stop=(et == ET - 1),
                            skip_group_check=True,
                        )
                # xsum += x16_s (exact: eye16 matmuls, psum fp32)
                for et in range(ET):
                    for qc in range(2):
                        nc.tensor.matmul(
                            xsump[:, et * QLOC + qc * 512 : et * QLOC + qc * 512 + 512],
                            lhsT=eye16_t[:],
                            rhs=x16_s[:, et, qc * 512 : qc * 512 + 512],
                            start=(s == 0),
                            stop=(s == S - 1),
                            skip_group_check=True,
                        )
            # qn = 0.25 * sum x^2   (x = -2(qt+b))
            nc.vector.tensor_scalar_mul(
                qncol_t[:].rearrange("p a b -> p (a b)"), qncolp[:], 0.25
            )
            # qnsum/(S-1) columns for the std bias (qncol already has the 0.25)
            nc.vector.tensor_reduce(
                qn9r_t[:], qncol_t[:], axis=mybir.AxisListType.X, op=ALU.add
            )
            nc.vector.tensor_scalar_mul(qn9_t[:], qn9r_t[:], 1.0 / (S - 1))
            nc.vector.tensor_copy(
                xsum16_t[:].rearrange("p a b -> p (a b)"), xsump[:]
            )

        # ---------- phase 2: distances, moments, outputs ----------
        with tc.tile_pool(name="ppC", bufs=2, space="PSUM") as ppC, \
             tc.tile_pool(name="maccpool", bufs=2) as maccpool:
            for qt8 in range(QT):
                for ph in range(NPH):
                    macc_t = maccpool.tile([128, PH], F32, tag="macc", name=f"m{qt8}_{ph}")
                    chains = [
                        ppC.tile([128, PH], F32, tag="ps", name=f"ch{qt8}_{ph}_{ab}")
                        for ab in range(2)
                    ]
                    for cp in chains:
                        for c in range(PH // 512):
                            o = ph * PH + c * 512
                            # pn seed: rank-1 ones x pn16q (fp16), once per chain
                            nc.tensor.matmul(
                                cp[:, c * 512 : c * 512 + 512],
                                lhsT=onesr16_t[:],
                                rhs=pn16q_t[:, o : o + 512],
                                start=True,
                                stop=True,
                                skip_group_check=True,
                            )
                    for s in range(S):
                        cp = chains[s % 2]
                        d16_s = x_tiles[s]
                        for et in range(ET):
                            for c in range(PH // 512):
                                o = ph * PH + c * 512
                                # delta cross accumulates onto the live chain
                                nc.tensor.matmul(
                                    cp[:, c * 512 : c * 512 + 512],
                                    lhsT=d16_s[:, et, qt8 * 128 : qt8 * 128 + 128],
                                    rhs=yT16_t[:, et, o : o + 512],
                                    start=False,
                                    stop=(et == ET - 1),
                                    skip_group_check=True,
                                )
                        # dist straight into macc for s=0, else via a rotating
                        # fp32 tile + exact DVE add (macc must be exact fp32:
                        # a PE f32r accumulation measured 1e-4 rel rounding,
                        # which the variance amplifies 360x -> std absmax 2.7)
                        dst = (
                            macc_t
                            if s == 0
                            else distpool.tile(
                                [128, PH], F32, tag="dist", name=f"d{qt8}_{ph}_{s}"
                            )
                        )
                        nc.scalar.activation(
                            dst[:], cp[:], AF.Sqrt,
                            bias=qncol_t[:, qt8, s : s + 1],
                            scale=1.0,
                        )
                        if s > 0:
                            nc.vector.tensor_add(macc_t[:], macc_t[:], dst[:])
                    # ss = 10*pn + xsum.proto^T (fp16 cross, consistent)
                    ssp = ppC.tile([128, PH], F32, tag="ps", name=f"ss{qt8}_{ph}")
                    for c in range(PH // 512):
                        o = ph * PH + c * 512
                        nc.tensor.matmul(
                            ssp[:, c * 512 : c * 512 + 512],
                            lhsT=onesr16_t[:],
                            rhs=pn10_t[:, o : o + 512],
                            start=True,
                            stop=False,
                            skip_group_check=True,
                        )
                    for et in range(ET):
                        for c in range(PH // 512):
                            o = ph * PH + c * 512
                            nc.tensor.matmul(
                                ssp[:, c * 512 : c * 512 + 512],
                                lhsT=xsum16_t[:, et, qt8 * 128 : qt8 * 128 + 128],
                                rhs=yT16_t[:, et, o : o + 512],
                                start=False,
                                stop=(et == ET - 1),
                                skip_group_check=True,
                            )
                    # omean = macc/S; m2 = omean^2; u = ss - m2*S;
                    # std = Sqrt(u/(S-1) + qnsum/(S-1)) via the ACT bias
                    omean_t = outpool.tile([128, PH], F32, tag="out", name=f"om{qt8}_{ph}")
                    nc.vector.tensor_scalar_mul(omean_t[:], macc_t[:], COMP / S)
                    m2_t = finpool.tile([128, PH], F32, tag="fin", name=f"m2{qt8}_{ph}")
                    nc.scalar.square(m2_t[:], omean_t[:])
                    u_t = finpool.tile([128, PH], F32, tag="fin", name=f"u{qt8}_{ph}")
                    nc.vector.scalar_tensor_tensor(
                        u_t[:], m2_t[:], -float(S), ssp[:], ALU.mult, ALU.add
                    )
                    ostd_t = outpool.tile([128, PH], F32, tag="out", name=f"os{qt8}_{ph}")
                    nc.scalar.activation(
                        ostd_t[:], u_t[:], AF.Sqrt,
                        bias=qn9_t[:, qt8 : qt8 + 1],
                        scale=1.0 / (S - 1),
                    )
                    nc.sync.dma_start(
                        std_o[qt8 * 128 : qt8 * 128 + 128, ph * PH : ph * PH + PH],
                        ostd_t[:],
                    )
                    nc.sync.dma_start(
                        mean_o[qt8 * 128 : qt8 * 128 + 128, ph * PH : ph * PH + PH],
                        omean_t[:],
                    )


def _prep_inputs(query_features, prototypes, weight_mu, weight_rho, bias_mu, bias_rho, eps_w, eps_b):
    f32, f16 = np.float32, np.float16
    sp_w = np.log1p(np.exp(weight_rho.astype(np.float64))).astype(f32)
    sp_b = np.log1p(np.exp(bias_rho.astype(np.float64))).astype(f32)
    W = (weight_mu[None] + eps_w * sp_w[None]).astype(f32)  # [S,D,D]
    B = (bias_mu[None] + eps_b * sp_b[None]).astype(f32)  # [S,D]
    Wh = W.astype(f16)
    qfh = query_features.astype(f16)  # [Q,D]

    yh = prototypes.astype(f16)  # [P,D]
    pn = (yh.astype(f32) ** 2).sum(-1, dtype=f32)  # [P]
    pn16q = pn.astype(f16)[None, :]  # [1,P] chain seed row
    pn10_16 = (float(S) * pn16q.astype(f32)).astype(f16)  # [1,P]
    b2 = (-2.0 * B).astype(f32)  # [S,D]

    W16 = np.ascontiguousarray(
        Wh.reshape(S, DT, 128, 256).transpose(0, 2, 1, 3).reshape(S, 128, DT * 256)
    )
    b2T = np.ascontiguousarray(
        b2.T.reshape(ET, 128, S).transpose(1, 0, 2).reshape(128, ET * S)
    )
    yT16 = np.ascontiguousarray(
        yh.T.reshape(ET, 128, P).transpose(1, 0, 2)
    )  # [128, ET, P]
    common = {
        "W16": W16,
        "b2T": b2T,
        "yT16": yT16,
        "pn16q": pn16q,
        "pn10_16": pn10_16,
        "onesr16": np.ones((1, 128), f16),
        "o16c": np.ones((128, 1), f16),
        "eye16": np.eye(128, dtype=f16),
    }
    in_maps = []
    for c in range(NCORES):
        qs = qfh[c * QLOC : (c + 1) * QLOC]  # [QLOC, D]
        qT16 = np.ascontiguousarray(
            qs.T.reshape(DT, 128, QLOC).transpose(1, 0, 2).reshape(128, DT * QLOC)
        )
        in_maps.append({"qT16": qT16, **common})
    return in_maps


def kernel(**inputs):
    global LAST_RESULTS
    n_samples = int(inputs.pop("n_samples", S))
    assert n_samples == S, f"kernel hardcodes S={S}, got {n_samples}"
    np_inputs = {
        k: np.asarray(v, dtype=np.float32)
        for k, v in inputs.items()
    }
    in_maps = _prep_inputs(**np_inputs)

    if "nc" not in _CACHE:
        _CACHE["nc"] = _build_bass()
    nc = _CACHE["nc"]

    trace = bool(int(os.environ.get("KERNEL_TRACE", "0")))
    res = bass_utils.run_bass_kernel_spmd(
        nc, in_maps, core_ids=list(range(NCORES)), trace=trace
    )
    LAST_RESULTS = res
    mean = np.concatenate([r["mean_o"] for r in res.results], axis=0)
    std = np.concatenate([r["std_o"] for r in res.results], axis=0)
    return mean, std


# revision 66
# speedup vs baseline: 1.4119x; 1.2564x over previous
"""Bayesian uncertainty distance kernel for TRN2 (8 NeuronCores, SPMD).

Math (per reference):
    W_s  = weight_mu + eps_w[s] * softplus(weight_rho)          [S,D,D]
    b_s  = bias_mu   + eps_b[s] * softplus(bias_rho)            [S,D]
    qt_s = query @ W_s + b_s                                    [S,Q,D]
    d2_s = ||qt_s||^2 - 2 qt_s.proto^T + ||proto||^2            [S,Q,P]
    mean = mean_s sqrt(d2_s);  std = std_s(sqrt(d2_s), ddof=1)

Sharding: data-parallel over Q (8192 -> 8 x 1024). Everything else replicated.

v3 design (per core, Q=1024, P=2048, D=256, S=10):
  - samples are DEFINED as x8_s := fp8e4(-2*(query@W_s + b_s)); all moments
    are derived consistently from these fp8 values so rounding cancels in
    the variance to first order.  prototypes y8 := fp8e4(proto); the pn
    norm and the fp16 copy yT16 are derived FROM y8 for exact consistency.
  - phase 1 per s: fp16 qt matmuls -> DVE tensor_scalar writes x8 directly
    (psum*-2 + -2b, fp8 out); x16u = up(x8) exact; x2 = x16u^2 (DVE fp16);
    qn columns via width-1 PE matmuls (lhsT=x2 128-col slices, rhs=ones col)
    accumulated into one [128, 80] psum tile; xsum psum += eye8 @ x8_s.
  - phase 2 per (qtile, phalf): per s a [128,1024] psum tile gets
    pn via a DoubleRow fp8 seed (ones8x [1,2,128] x pn8x [1,2,P], value
    2*fp8(pn/2)) + the cross term via DoubleRow fp8 matmuls (K=256 in one
    instr, 0.5 cyc/col); dist_s = ACT Sqrt(psum + qn_s column bias) ->
    fp32 SBUF; the s-sum runs on the PE as identity-matmul accumulation
    with float32r (1 cyc/col, fp22 truncation compensated by COMP scale)
    instead of DVE fp32 adds (1x-mode, was the 165us DVE bottleneck).
  - variance via sum-of-d2: ss psum = ones x 10pn16 + xsum16 @ yT16 (fp16);
    u = ss - macc^2*COMP^2/10 (DVE stt); std = Sqrt(u/(S-1) + qnsum/(S-1))
    with the qnsum column as the ACT bias; mean = macc*COMP/10 (DVE ts).

The host does only O(S*D^2) prep in numpy (softplus, W_s, transposes, pn).
"""

import os
import numpy as np
import ml_dtypes

import concourse.bass as bass
import concourse.mybir as mybir
import concourse.tile as tile
from concourse import bacc, bass_utils

AF = mybir.ActivationFunctionType
ALU = mybir.AluOpType
DR = mybir.MatmulPerfMode.DoubleRow

F32 = mybir.dt.float32
F32R = mybir.dt.float32r
F16 = mybir.dt.float16
F8 = mybir.dt.float8e4
NP_F8 = ml_dtypes.float8_e4m3  # TRN float8e4: max normal +-240, has inf

NCORES = 8
D = 256
Q_FULL = 8192
P = 2048
S = 10
QLOC = Q_FULL // NCORES  # 1024
ET = D // 128  # 2 e-tiles
DT = D // 128  # 2 d-tiles
QT = QLOC // 128  # 8 q-tiles per core
PH = 2048  # phase-2 psum tile width (4 banks)
NPH = P // PH  # 1

# fp22 truncation compensation: the PE reads float32r by truncating the
# mantissa to 13 bits, losing an average of 2^-14 relative on the positive
# dist values; COMP re-centres macc (validated against the reference).
COMP = 1.0

_CACHE = {}
LAST_RESULTS = None


def _build_bass():
    nc = bacc.Bacc(
        "TRN2",
        target_bir_lowering=False,
        debug=False,
        num_devices=NCORES,
    )
    ins = {}
    ins["qT16"] = nc.dram_tensor("qT16", [128, DT * QLOC], F16, kind="ExternalInput").ap()
    ins["W16"] = nc.dram_tensor("W16", [S, 128, DT * 256], F16, kind="ExternalInput").ap()
    ins["b2T"] = nc.dram_tensor("b2T", [128, ET * S], F32, kind="ExternalInput").ap()
    ins["yT16"] = nc.dram_tensor("yT16", [128, ET, P], F16, kind="ExternalInput").ap()
    ins["pn16q"] = nc.dram_tensor("pn16q", [1, P], F16, kind="ExternalInput").ap()
    ins["pn10_16"] = nc.dram_tensor("pn10_16", [1, P], F16, kind="ExternalInput").ap()
    ins["onesr16"] = nc.dram_tensor("onesr16", [1, 128], F16, kind="ExternalInput").ap()
    ins["o16c"] = nc.dram_tensor("o16c", [128, 1], F16, kind="ExternalInput").ap()
    ins["eye16"] = nc.dram_tensor("eye16", [128, 128], F16, kind="ExternalInput").ap()
    mean_o = nc.dram_tensor("mean_o", [QLOC, P], F32, kind="ExternalOutput").ap()
    std_o = nc.dram_tensor("std_o", [QLOC, P], F32, kind="ExternalOutput").ap()

    with tile.TileContext(nc) as tc:
        _kernel_body(tc, ins, mean_o, std_o)
    nc.compile()
    return nc


def _kernel_body(tc, ins, mean_o, std_o):
    nc = tc.nc
    from contextlib import ExitStack

    ctx = ExitStack()
    with ctx:
        cpool = ctx.enter_context(tc.tile_pool(name="consts", bufs=1))
        wpool = ctx.enter_context(tc.tile_pool(name="wpool", bufs=2))
        dpool = ctx.enter_context(tc.tile_pool(name="dpool", bufs=S))
        x16pool = ctx.enter_context(tc.tile_pool(name="x16p", bufs=3))
        x2pool = ctx.enter_context(tc.tile_pool(name="x2pool", bufs=2))
        xsumpool = ctx.enter_context(tc.tile_pool(name="xsumpool", bufs=1))
        qnpool = ctx.enter_context(tc.tile_pool(name="qnpool", bufs=1))
        distpool = ctx.enter_context(tc.tile_pool(name="distpool", bufs=4))
        finpool = ctx.enter_context(tc.tile_pool(name="finpool", bufs=2))
        outpool = ctx.enter_context(tc.tile_pool(name="outpool", bufs=3))

        # ---- constants into SBUF ----
        qT_t = cpool.tile([128, DT * QLOC], F16)
        nc.sync.dma_start(qT_t[:], ins["qT16"])
        b2_t = cpool.tile([128, ET * S], F32)
        nc.sync.dma_start(b2_t[:], ins["b2T"])
        yT16_t = cpool.tile([128, ET, P], F16)
        nc.sync.dma_start(yT16_t[:], ins["yT16"])
        pn16q_t = cpool.tile([1, P], F16)
        nc.sync.dma_start(pn16q_t[:], ins["pn16q"])
        pn10_t = cpool.tile([1, P], F16)
        nc.sync.dma_start(pn10_t[:], ins["pn10_16"])
        onesr16_t = cpool.tile([1, 128], F16)
        nc.sync.dma_start(onesr16_t[:], ins["onesr16"])
        o16c_t = cpool.tile([128, 1], F16)
        nc.sync.dma_start(o16c_t[:], ins["o16c"])
        eye16_t = cpool.tile([128, 128], F16)
        nc.sync.dma_start(eye16_t[:], ins["eye16"])

        xsum16_t = xsumpool.tile([128, ET, QLOC], F16)
        # qn columns: [128, QT, S] fp32; [128,1] slices feed the ACT Sqrt bias
        qncol_t = qnpool.tile([128, QT, S], F32)
        qn9r_t = qnpool.tile([128, QT], F32)
        qn9_t = qnpool.tile([128, QT], F32)  # qnsum/(S-1) bias columns for std

        x_tiles = []
        # ---------- phase 1: per-sample fp8 transformed queries ----------
        with tc.tile_pool(name="pp1", bufs=2, space="PSUM") as pp1, \
             tc.tile_pool(name="ppqn", bufs=1, space="PSUM") as ppqn, \
             tc.tile_pool(name="ppxs", bufs=1, space="PSUM") as ppxs:
            qncolp = ppqn.tile([128, QT * S], F32)
            xsump = ppxs.tile([128, ET * QLOC], F32)
            x16_list = []
            for s in range(S):
                w_t = wpool.tile([128, DT * 256], F16, tag="w")
                nc.sync.dma_start(w_t[:], ins["W16"][s])
                # s<2 tiles start the two chains and must survive into
                # phase 2, so they come from the persistent delta pool
                if s < 2:
                    x16_s = dpool.tile([128, ET, QLOC], F16, tag="d16", name=f"x16_{s}")
                else:
                    x16_s = x16pool.tile([128, ET, QLOC], F16, tag="x16", name=f"x16_{s}")
                x16_list.append(x16_s)
                for et in range(ET):
                    for qc in range(2):
                        qp = pp1.tile([128, 512], F32, tag="ps", name=f"qp{s}_{et}_{qc}")
                        for dt_ in range(DT):
                            nc.tensor.matmul(
                                qp[:],
                                lhsT=w_t[:, dt_ * 256 + et * 128 : dt_ * 256 + et * 128 + 128],
                                rhs=qT_t[:, dt_ * QLOC + qc * 512 : dt_ * QLOC + qc * 512 + 512],
                                start=(dt_ == 0),
                                stop=(dt_ == DT - 1),
                            )
                        # x16 = fp16(-2*qt - 2*b) from psum on DVE
                        nc.vector.tensor_scalar(
                            x16_s[:, et, qc * 512 : qc * 512 + 512],
                            qp[:],
                            -2.0,
                            b2_t[:, et * S + s : et * S + s + 1],
                            ALU.mult,
                            ALU.add,
                        )
                # delta chains over stride-2 samples: the phase-2 psum keeps
                # pn + x_path.y alive across the chain, so only the delta is
                # multiplied each step (no per-sample rank-2 reseeding, which
                # was ~25% of all PE matmul issue slots).  fp16 deltas round
                # at ~2^-11|delta| per step, small enough for the std (fp8
                # deltas measured 2.3e-2 std error; fp16 keeps it at ~4e-3).
                if s < 2:
                    d16_s = x16_s
                else:
                    d16_s = dpool.tile([128, ET, QLOC], F16, tag="d16", name=f"dd{s}")
                    nc.vector.tensor_tensor(
                        d16_s[:], x16_s[:], x16_list[s - 2][:], ALU.subtract
                    )
                x_tiles.append(d16_s)
                x2_s = x2pool.tile([128, ET, QLOC], F16, tag="x2", name=f"x2_{s}")
                nc.scalar.square(x2_s[:], x16_s[:])
                # qn columns: width-1 matmuls, one column per (qtile, s)
                for qt8 in range(QT):
                    for et in range(ET):
                        nc.tensor.matmul(
                            qncolp[:, qt8 * S + s : qt8 * S + s + 1],
                            lhsT=x2_s[:, et, qt8 * 128 : qt8 * 128 + 128],
                            rhs=o16c_t[:],
                            start=(et == 0),
                            stop=(et == ET - 1),
                            skip_group_check=True,
                        )
                # xsum += x16_s (exact: eye16 matmuls, psum fp32)
                for et in range(ET):
                    for qc in range(2):
                        nc.tensor.matmul(
                            xsump[:, et * QLOC + qc * 512 : et * QLOC + qc * 512 + 512],
                            lhsT=eye16_t[:],
                            rhs=x16_s[:, et, qc * 512 : qc * 512 + 512],
                            start=(s == 0),
                            stop=(s == S - 1),
                            skip_group_check=True,
                        )
            # qn = 0.25 * sum x^2   (x = -2(qt+b))
            nc.vector.tensor_scalar_mul(
                qncol_t[:].rearrange("p a b -> p (a b)"), qncolp[:], 0.25
            )
            # qnsum/(S-1) columns for the std bias (qncol already has the 0.25)
            nc.vector.tensor_reduce(
                qn9r_t[:], qncol_t[:], axis=mybir.AxisListType.X, op=ALU.add
            )
            nc.vector.tensor_scalar_mul(qn9_t[:], qn9r_t[:], 1.0 / (S - 1))
            nc.vector.tensor_copy(
                xsum16_t[:].rearrange("p a b -> p (a b)"), xsump[:]
            )

        # ---------- phase 2: distances, moments, outputs ----------
        with tc.tile_pool(name="ppC", bufs=4, space="PSUM") as ppC, \
             tc.tile_pool(name="maccpool", bufs=2) as maccpool:
            for qt8 in range(QT):
                for ph in range(NPH):
                    macc_t = maccpool.tile([128, PH], F32, tag="macc", name=f"m{qt8}_{ph}")
                    # 4 half-width chains (A/B samples x lo/hi p-halves): same
                    # matmul+LDW count as 2 full-width chains but twice the
                    # independent psum buffers, so the per-chain PE->ACT->PE
                    # WAR serialization overlaps across halves
                    chains = [
                        ppC.tile([128, PH // 2], F32, tag="ps", name=f"ch{qt8}_{ph}_{ab}")
                        for ab in range(4)
                    ]
                    for ci, cp in enumerate(chains):
                        hb = (ci // 2) * (PH // 2)
                        for c in range(PH // 1024):
                            o = ph * PH + hb + c * 512
                            # pn seed: rank-1 ones x pn16q (fp16), once per chain
                            nc.tensor.matmul(
                                cp[:, c * 512 : c * 512 + 512],
                                lhsT=onesr16_t[:],
                                rhs=pn16q_t[:, o : o + 512],
                                start=True,
                                stop=True,
                                skip_group_check=True,
                            )
                    for s in range(S):
                        d16_s = x_tiles[s]
                        for et in range(ET):
                            lhs = d16_s[:, et, qt8 * 128 : qt8 * 128 + 128]
                            for hf in range(2):
                                cp = chains[s % 2 + 2 * hf]
                                for c in range(PH // 1024):
                                    o = ph * PH + hf * (PH // 2) + c * 512
                                    # delta cross accumulates onto the live chain
                                    nc.tensor.matmul(
                                        cp[:, c * 512 : c * 512 + 512],
                                        lhsT=lhs,
                                        rhs=yT16_t[:, et, o : o + 512],
                                        start=False,
                                        stop=(et == ET - 1),
                                        skip_group_check=True,
                                    )
                        # dist straight into macc for s=0, else via a rotating
                        # fp32 tile + exact DVE add (macc must be exact fp32:
                        # a PE f32r accumulation measured 1e-4 rel rounding,
                        # which the variance amplifies 360x -> std absmax 2.7)
                        dst = (
                            macc_t
                            if s == 0
                            else distpool.tile(
                                [128, PH], F32, tag="dist", name=f"d{qt8}_{ph}_{s}"
                            )
                        )
                        for hf in range(2):
                            nc.scalar.activation(
                                dst[:, hf * (PH // 2) : (hf + 1) * (PH // 2)],
                                chains[s % 2 + 2 * hf][:],
                                AF.Sqrt,
                                bias=qncol_t[:, qt8, s : s + 1],
                                scale=1.0,
                            )
                        if s > 0:
                            nc.vector.tensor_add(macc_t[:], macc_t[:], dst[:])
                    # ss = 10*pn + xsum.proto^T (fp16 cross, consistent)
                    ssps = [
                        ppC.tile([128, PH // 2], F32, tag="ps", name=f"ss{qt8}_{ph}_{hf}")
                        for hf in range(2)
                    ]
                    for hf in range(2):
                        for c in range(PH // 1024):
                            o = ph * PH + hf * (PH // 2) + c * 512
                            nc.tensor.matmul(
                                ssps[hf][:, c * 512 : c * 512 + 512],
                                lhsT=onesr16_t[:],
                                rhs=pn10_t[:, o : o + 512],
                                start=True,
                                stop=False,
                                skip_group_check=True,
                            )
                    for et in range(ET):
                        lhs = xsum16_t[:, et, qt8 * 128 : qt8 * 128 + 128]
                        for hf in range(2):
                            for c in range(PH // 1024):
                                o = ph * PH + hf * (PH // 2) + c * 512
                                nc.tensor.matmul(
                                    ssps[hf][:, c * 512 : c * 512 + 512],
                                    lhsT=lhs,
                                    rhs=yT16_t[:, et, o : o + 512],
                                    start=False,
                                    stop=(et == ET - 1),
                                    skip_group_check=True,
                                )
                    # omean = macc/S; m2 = omean^2; u = ss - m2*S;
                    # std = Sqrt(u/(S-1) + qnsum/(S-1)) via the ACT bias
                    omean_t = outpool.tile([128, PH], F32, tag="out", name=f"om{qt8}_{ph}")
                    nc.vector.tensor_scalar_mul(omean_t[:], macc_t[:], COMP / S)
                    m2_t = finpool.tile([128, PH], F32, tag="fin", name=f"m2{qt8}_{ph}")
                    nc.scalar.square(m2_t[:], omean_t[:])
                    u_t = finpool.tile([128, PH], F32, tag="fin", name=f"u{qt8}_{ph}")
                    for hf in range(2):
                        sl = slice(hf * (PH // 2), (hf + 1) * (PH // 2))
                        nc.vector.scalar_tensor_tensor(
                            u_t[:, sl], m2_t[:, sl], -float(S), ssps[hf][:],
                            ALU.mult, ALU.add,
                        )
                    ostd_t = outpool.tile([128, PH], F32, tag="out", name=f"os{qt8}_{ph}")
                    nc.scalar.activation(
                        ostd_t[:], u_t[:], AF.Sqrt,
                        bias=qn9_t[:, qt8 : qt8 + 1],
                        scale=1.0 / (S - 1),
                    )
                    nc.sync.dma_start(
                        std_o[qt8 * 128 : qt8 * 128 + 128, ph * PH : ph * PH + PH],
                        ostd_t[:],
                    )
                    nc.sync.dma_start(
                        mean_o[qt8 * 128 : qt8 * 128 + 128, ph * PH : ph * PH + PH],
                        omean_t[:],
                    )


def _prep_inputs(query_features, prototypes, weight_mu, weight_rho, bias_mu, bias_rho, eps_w, eps_b):
    f32, f16 = np.float32, np.float16
    sp_w = np.log1p(np.exp(weight_rho.astype(np.float64))).astype(f32)
    sp_b = np.log1p(np.exp(bias_rho.astype(np.float64))).astype(f32)
    W = (weight_mu[None] + eps_w * sp_w[None]).astype(f32)  # [S,D,D]
    B = (bias_mu[None] + eps_b * sp_b[None]).astype(f32)  # [S,D]
    Wh = W.astype(f16)
    qfh = query_features.astype(f16)  # [Q,D]

    yh = prototypes.astype(f16)  # [P,D]
    pn = (yh.astype(f32) ** 2).sum(-1, dtype=f32)  # [P]
    pn16q = pn.astype(f16)[None, :]  # [1,P] chain seed row
    pn10_16 = (float(S) * pn16q.astype(f32)).astype(f16)  # [1,P]
    b2 = (-2.0 * B).astype(f32)  # [S,D]

    W16 = np.ascontiguousarray(
        Wh.reshape(S, DT, 128, 256).transpose(0, 2, 1, 3).reshape(S, 128, DT * 256)
    )
    b2T = np.ascontiguousarray(
        b2.T.reshape(ET, 128, S).transpose(1, 0, 2).reshape(128, ET * S)
    )
    yT16 = np.ascontiguousarray(
        yh.T.reshape(ET, 128, P).transpose(1, 0, 2)
    )  # [128, ET, P]
    common = {
        "W16": W16,
        "b2T": b2T,
        "yT16": yT16,
        "pn16q": pn16q,
        "pn10_16": pn10_16,
        "onesr16": np.ones((1, 128), f16),
        "o16c": np.ones((128, 1), f16),
        "eye16": np.eye(128, dtype=f16),
    }
    in_maps = []
    for c in range(NCORES):
        qs = qfh[c * QLOC : (c + 1) * QLOC]  # [QLOC, D]
        qT16 = np.ascontiguousarray(
            qs.T.reshape(DT, 128, QLOC).transpose(1, 0, 2).reshape(128, DT * QLOC)
        )
        in_maps.append({"qT16": qT16, **common})
    return in_maps


def kernel(**inputs):
    global LAST_RESULTS
    n_samples = int(inputs.pop("n_samples", S))
    assert n_samples == S, f"kernel hardcodes S={S}, got {n_samples}"
    np_inputs = {
        k: np.asarray(v, dtype=np.float32)
        for k, v in inputs.items()
    }
    in_maps = _prep_inputs(**np_inputs)

    if "nc" not in _CACHE:
        _CACHE["nc"] = _build_bass()
    nc = _CACHE["nc"]

    trace = bool(int(os.environ.get("KERNEL_TRACE", "0")))
    res = bass_utils.run_bass_kernel_spmd(
        nc, in_maps, core_ids=list(range(NCORES)), trace=trace
    )
    LAST_RESULTS = res
    mean = np.concatenate([r["mean_o"] for r in res.results], axis=0)
    std = np.concatenate([r["std_o"] for r in res.results], axis=0)
    return mean, std


# revision 69
# speedup vs baseline: 1.4148x; 1.0020x over previous
"""Bayesian uncertainty distance kernel for TRN2 (8 NeuronCores, SPMD).

Math (per reference):
    W_s  = weight_mu + eps_w[s] * softplus(weight_rho)          [S,D,D]
    b_s  = bias_mu   + eps_b[s] * softplus(bias_rho)            [S,D]
    qt_s = query @ W_s + b_s                                    [S,Q,D]
    d2_s = ||qt_s||^2 - 2 qt_s.proto^T + ||proto||^2            [S,Q,P]
    mean = mean_s sqrt(d2_s);  std = std_s(sqrt(d2_s), ddof=1)

Sharding: data-parallel over Q (8192 -> 8 x 1024). Everything else replicated.

Design (per core, Q=1024, P=2048, D=256, S=10), ~342us measured:
  - samples are DEFINED as x_s := fp16(-2*(query@W_s + b_s)); every moment
    derives consistently from these values so rounding cancels in the
    variance to first order.
  - phase 1 per s: fp16 qt matmuls -> x_s = DVE tensor_scalar(psum*-2-2b);
    delta d_s = x_s - x_{s-2} (DVE fp16; drift ~2^-11|d| per step -- fp8
    deltas measured 2.3e-2 std error, fp16 keeps it at 6.7e-3);
    x2 = ACT Square(x_s); qn COLUMNS via width-1 PE matmuls (lhsT = x2
    128-col slices, rhs = ones col) into one [128, 80] psum tile -- they
    feed the phase-2 Sqrt as per-partition biases, which is what lets the
    rank-2 qn reseeding (25% of baseline PE issue slots) disappear;
    xsum psum += eye16 @ x_s.
  - phase 2 per qtile: FOUR [128,1024] psum chains (even/odd samples x
    lo/hi p-halves).  Each chain is seeded once with the rank-1 pn row,
    then per sample only the delta cross accumulates (K=2x128 fp16,
    start=False groups).  4 chains (vs 2 full-width) double the psum
    buffering so the per-chain PE->ACT WAR serialization overlaps; this
    was worth 88us of span.  dist = ACT Sqrt(chain + qn_s bias);
    macc += dist on DVE in exact fp32 (a PE float32r identity-matmul
    accumulation measured 1e-4 rel rounding which the variance amplifies
    ~360x into 0.16 std rms -- unusable).  fp8 DoubleRow crosses measured
    ~380ns/instr vs fp16's ~216ns on this stack (pessimization, reverted).
  - variance via sum-of-d2: ss = rank-1 ones x 10pn + xsum16 @ yT16;
    u = ss - (macc/S)^2*S (m2 on ACT Square); std = Sqrt(u/(S-1) + qn9
    bias) where qn9 = qnsum/(S-1) columns.  mean = macc/S (DVE).

The host does only O(S*D^2) prep in numpy (softplus, W_s, transposes, pn).
"""

import os
import numpy as np

import concourse.bass as bass
import concourse.mybir as mybir
import concourse.tile as tile
from concourse import bacc, bass_utils

AF = mybir.ActivationFunctionType
ALU = mybir.AluOpType

F32 = mybir.dt.float32
F16 = mybir.dt.float16

NCORES = 8
D = 256
Q_FULL = 8192
P = 2048
S = 10
QLOC = Q_FULL // NCORES  # 1024
ET = D // 128  # 2 e-tiles
DT = D // 128  # 2 d-tiles
QT = QLOC // 128  # 8 q-tiles per core
PH = 2048  # phase-2 psum tile width (4 banks)
NPH = P // PH  # 1



_CACHE = {}
LAST_RESULTS = None


def _build_bass():
    nc = bacc.Bacc(
        "TRN2",
        target_bir_lowering=False,
        debug=False,
        num_devices=NCORES,
    )
    ins = {}
    ins["qT16"] = nc.dram_tensor("qT16", [128, DT * QLOC], F16, kind="ExternalInput").ap()
    ins["W16"] = nc.dram_tensor("W16", [S, 128, DT * 256], F16, kind="ExternalInput").ap()
    ins["b2T"] = nc.dram_tensor("b2T", [128, ET * S], F32, kind="ExternalInput").ap()
    ins["yT16"] = nc.dram_tensor("yT16", [128, ET, P], F16, kind="ExternalInput").ap()
    ins["pn16q"] = nc.dram_tensor("pn16q", [1, P], F16, kind="ExternalInput").ap()
    ins["pn10_16"] = nc.dram_tensor("pn10_16", [1, P], F16, kind="ExternalInput").ap()
    ins["onesr16"] = nc.dram_tensor("onesr16", [1, 128], F16, kind="ExternalInput").ap()
    ins["o16c"] = nc.dram_tensor("o16c", [128, 1], F16, kind="ExternalInput").ap()
    ins["eye16"] = nc.dram_tensor("eye16", [128, 128], F16, kind="ExternalInput").ap()
    mean_o = nc.dram_tensor("mean_o", [QLOC, P], F32, kind="ExternalOutput").ap()
    std_o = nc.dram_tensor("std_o", [QLOC, P], F32, kind="ExternalOutput").ap()

    with tile.TileContext(nc) as tc:
        _kernel_body(tc, ins, mean_o, std_o)
    nc.compile()
    return nc


def _kernel_body(tc, ins, mean_o, std_o):
    nc = tc.nc
    from contextlib import ExitStack

    ctx = ExitStack()
    with ctx:
        cpool = ctx.enter_context(tc.tile_pool(name="consts", bufs=1))
        wpool = ctx.enter_context(tc.tile_pool(name="wpool", bufs=2))
        dpool = ctx.enter_context(tc.tile_pool(name="dpool", bufs=S))
        x16pool = ctx.enter_context(tc.tile_pool(name="x16p", bufs=3))
        x2pool = ctx.enter_context(tc.tile_pool(name="x2pool", bufs=2))
        xsumpool = ctx.enter_context(tc.tile_pool(name="xsumpool", bufs=1))
        qnpool = ctx.enter_context(tc.tile_pool(name="qnpool", bufs=1))
        distpool = ctx.enter_context(tc.tile_pool(name="distpool", bufs=4))
        finpool = ctx.enter_context(tc.tile_pool(name="finpool", bufs=2))
        outpool = ctx.enter_context(tc.tile_pool(name="outpool", bufs=3))

        # ---- constants into SBUF ----
        qT_t = cpool.tile([128, DT * QLOC], F16)
        nc.sync.dma_start(qT_t[:], ins["qT16"])
        b2_t = cpool.tile([128, ET * S], F32)
        nc.sync.dma_start(b2_t[:], ins["b2T"])
        yT16_t = cpool.tile([128, ET, P], F16)
        nc.sync.dma_start(yT16_t[:], ins["yT16"])
        pn16q_t = cpool.tile([1, P], F16)
        nc.sync.dma_start(pn16q_t[:], ins["pn16q"])
        pn10_t = cpool.tile([1, P], F16)
        nc.sync.dma_start(pn10_t[:], ins["pn10_16"])
        onesr16_t = cpool.tile([1, 128], F16)
        nc.sync.dma_start(onesr16_t[:], ins["onesr16"])
        o16c_t = cpool.tile([128, 1], F16)
        nc.sync.dma_start(o16c_t[:], ins["o16c"])
        eye16_t = cpool.tile([128, 128], F16)
        nc.sync.dma_start(eye16_t[:], ins["eye16"])

        xsum16_t = xsumpool.tile([128, ET, QLOC], F16)
        # qn columns: [128, QT, S] fp32; [128,1] slices feed the ACT Sqrt bias
        qncol_t = qnpool.tile([128, QT, S], F32)
        qn9r_t = qnpool.tile([128, QT], F32)
        qn9_t = qnpool.tile([128, QT], F32)  # qnsum/(S-1) bias columns for std

        x_tiles = []
        # ---------- phase 1: per-sample transformed queries + deltas ----------
        with tc.tile_pool(name="pp1", bufs=2, space="PSUM") as pp1, \
             tc.tile_pool(name="ppqn", bufs=1, space="PSUM") as ppqn, \
             tc.tile_pool(name="ppxs", bufs=1, space="PSUM") as ppxs:
            qncolp = ppqn.tile([128, QT * S], F32)
            xsump = ppxs.tile([128, ET * QLOC], F32)
            x16_list = []
            for s in range(S):
                w_t = wpool.tile([128, DT * 256], F16, tag="w")
                nc.sync.dma_start(w_t[:], ins["W16"][s])
                # s<2 tiles start the chains and must survive into
                # phase 2, so they come from the persistent delta pool
                if s < 2:
                    x16_s = dpool.tile([128, ET, QLOC], F16, tag="d16", name=f"x16_{s}")
                else:
                    x16_s = x16pool.tile([128, ET, QLOC], F16, tag="x16", name=f"x16_{s}")
                x16_list.append(x16_s)
                for et in range(ET):
                    for qc in range(2):
                        qp = pp1.tile([128, 512], F32, tag="ps", name=f"qp{s}_{et}_{qc}")
                        for dt_ in range(DT):
                            nc.tensor.matmul(
                                qp[:],
                                lhsT=w_t[:, dt_ * 256 + et * 128 : dt_ * 256 + et * 128 + 128],
                                rhs=qT_t[:, dt_ * QLOC + qc * 512 : dt_ * QLOC + qc * 512 + 512],
                                start=(dt_ == 0),
                                stop=(dt_ == DT - 1),
                            )
                        # x16 = fp16(-2*qt - 2*b) from psum on DVE
                        nc.vector.tensor_scalar(
                            x16_s[:, et, qc * 512 : qc * 512 + 512],
                            qp[:],
                            -2.0,
                            b2_t[:, et * S + s : et * S + s + 1],
                            ALU.mult,
                            ALU.add,
                        )
                # delta chains over stride-2 samples: the phase-2 psum keeps
                # pn + x_path.y alive across the chain, so only the delta is
                # multiplied each step (no per-sample rank-2 reseeding, which
                # was ~25% of all PE matmul issue slots).  fp16 deltas round
                # at ~2^-11|delta| per step, small enough for the std (fp8
                # deltas measured 2.3e-2 std error; fp16 keeps it at ~4e-3).
                if s < 2:
                    d16_s = x16_s
                else:
                    d16_s = dpool.tile([128, ET, QLOC], F16, tag="d16", name=f"dd{s}")
                    nc.vector.tensor_tensor(
                        d16_s[:], x16_s[:], x16_list[s - 2][:], ALU.subtract
                    )
                x_tiles.append(d16_s)
                x2_s = x2pool.tile([128, ET, QLOC], F16, tag="x2", name=f"x2_{s}")
                nc.scalar.square(x2_s[:], x16_s[:])
                # qn columns: width-1 matmuls, one column per (qtile, s)
                for qt8 in range(QT):
                    for et in range(ET):
                        nc.tensor.matmul(
                            qncolp[:, qt8 * S + s : qt8 * S + s + 1],
                            lhsT=x2_s[:, et, qt8 * 128 : qt8 * 128 + 128],
                            rhs=o16c_t[:],
                            start=(et == 0),
                            stop=(et == ET - 1),
                            skip_group_check=True,
                        )
                # xsum += x16_s (exact: eye16 matmuls, psum fp32)
                for et in range(ET):
                    for qc in range(2):
                        nc.tensor.matmul(
                            xsump[:, et * QLOC + qc * 512 : et * QLOC + qc * 512 + 512],
                            lhsT=eye16_t[:],
                            rhs=x16_s[:, et, qc * 512 : qc * 512 + 512],
                            start=(s == 0),
                            stop=(s == S - 1),
                            skip_group_check=True,
                        )
            # qn = 0.25 * sum x^2   (x = -2(qt+b))
            nc.vector.tensor_scalar_mul(
                qncol_t[:].rearrange("p a b -> p (a b)"), qncolp[:], 0.25
            )
            # qnsum/(S-1) columns for the std bias (qncol already has the 0.25)
            nc.vector.tensor_reduce(
                qn9r_t[:], qncol_t[:], axis=mybir.AxisListType.X, op=ALU.add
            )
            nc.vector.tensor_scalar_mul(qn9_t[:], qn9r_t[:], 1.0 / (S - 1))
            nc.vector.tensor_copy(
                xsum16_t[:].rearrange("p a b -> p (a b)"), xsump[:]
            )

        # ---------- phase 2: distances, moments, outputs ----------
        with tc.tile_pool(name="ppC", bufs=4, space="PSUM") as ppC, \
             tc.tile_pool(name="maccpool", bufs=2) as maccpool:
            for qt8 in range(QT):
                for ph in range(NPH):
                    macc_t = maccpool.tile([128, PH], F32, tag="macc", name=f"m{qt8}_{ph}")
                    # 4 half-width chains (A/B samples x lo/hi p-halves): same
                    # matmul+LDW count as 2 full-width chains but twice the
                    # independent psum buffers, so the per-chain PE->ACT->PE
                    # WAR serialization overlaps across halves
                    chains = [
                        ppC.tile([128, PH // 2], F32, tag="ps", name=f"ch{qt8}_{ph}_{ab}")
                        for ab in range(4)
                    ]
                    for ci, cp in enumerate(chains):
                        hb = (ci // 2) * (PH // 2)
                        for c in range(PH // 1024):
                            o = ph * PH + hb + c * 512
                            # pn seed: rank-1 ones x pn16q (fp16), once per chain
                            nc.tensor.matmul(
                                cp[:, c * 512 : c * 512 + 512],
                                lhsT=onesr16_t[:],
                                rhs=pn16q_t[:, o : o + 512],
                                start=True,
                                stop=True,
                                skip_group_check=True,
                            )
                    for s in range(S):
                        d16_s = x_tiles[s]
                        for et in range(ET):
                            lhs = d16_s[:, et, qt8 * 128 : qt8 * 128 + 128]
                            for hf in range(2):
                                cp = chains[s % 2 + 2 * hf]
                                for c in range(PH // 1024):
                                    o = ph * PH + hf * (PH // 2) + c * 512
                                    # delta cross accumulates onto the live chain
                                    nc.tensor.matmul(
                                        cp[:, c * 512 : c * 512 + 512],
                                        lhsT=lhs,
                                        rhs=yT16_t[:, et, o : o + 512],
                                        start=False,
                                        stop=(et == ET - 1),
                                        skip_group_check=True,
                                    )
                        # dist straight into macc for s=0, else via a rotating
                        # fp32 tile + exact DVE add (macc must be exact fp32:
                        # a PE f32r accumulation measured 1e-4 rel rounding,
                        # which the variance amplifies 360x -> std absmax 2.7)
                        dst = (
                            macc_t
                            if s == 0
                            else distpool.tile(
                                [128, PH], F32, tag="dist", name=f"d{qt8}_{ph}_{s}"
                            )
                        )
                        for hf in range(2):
                            nc.scalar.activation(
                                dst[:, hf * (PH // 2) : (hf + 1) * (PH // 2)],
                                chains[s % 2 + 2 * hf][:],
                                AF.Sqrt,
                                bias=qncol_t[:, qt8, s : s + 1],
                                scale=1.0,
                            )
                        if s > 0:
                            nc.vector.tensor_add(macc_t[:], macc_t[:], dst[:])
                    # ss = 10*pn + xsum.proto^T (fp16 cross, consistent)
                    ssps = [
                        ppC.tile([128, PH // 2], F32, tag="ps", name=f"ss{qt8}_{ph}_{hf}")
                        for hf in range(2)
                    ]
                    for hf in range(2):
                        for c in range(PH // 1024):
                            o = ph * PH + hf * (PH // 2) + c * 512
                            nc.tensor.matmul(
                                ssps[hf][:, c * 512 : c * 512 + 512],
                                lhsT=onesr16_t[:],
                                rhs=pn10_t[:, o : o + 512],
                                start=True,
                                stop=False,
                                skip_group_check=True,
                            )
                    for et in range(ET):
                        lhs = xsum16_t[:, et, qt8 * 128 : qt8 * 128 + 128]
                        for hf in range(2):
                            for c in range(PH // 1024):
                                o = ph * PH + hf * (PH // 2) + c * 512
                                nc.tensor.matmul(
                                    ssps[hf][:, c * 512 : c * 512 + 512],
                                    lhsT=lhs,
                                    rhs=yT16_t[:, et, o : o + 512],
                                    start=False,
                                    stop=(et == ET - 1),
                                    skip_group_check=True,
                                )
                    # omean = macc/S; m2 = omean^2; u = ss - m2*S;
                    # std = Sqrt(u/(S-1) + qnsum/(S-1)) via the ACT bias
                    omean_t = outpool.tile([128, PH], F32, tag="out", name=f"om{qt8}_{ph}")
                    nc.vector.tensor_scalar_mul(omean_t[:], macc_t[:], 1.0 / S)
                    m2_t = finpool.tile([128, PH], F32, tag="fin", name=f"m2{qt8}_{ph}")
                    nc.scalar.square(m2_t[:], omean_t[:])
                    u_t = finpool.tile([128, PH], F32, tag="fin", name=f"u{qt8}_{ph}")
                    for hf in range(2):
                        sl = slice(hf * (PH // 2), (hf + 1) * (PH // 2))
                        nc.vector.scalar_tensor_tensor(
                            u_t[:, sl], m2_t[:, sl], -float(S), ssps[hf][:],
                            ALU.mult, ALU.add,
                        )
                    ostd_t = outpool.tile([128, PH], F32, tag="out", name=f"os{qt8}_{ph}")
                    nc.scalar.activation(
                        ostd_t[:], u_t[:], AF.Sqrt,
                        bias=qn9_t[:, qt8 : qt8 + 1],
                        scale=1.0 / (S - 1),
                    )
                    nc.sync.dma_start(
                        std_o[qt8 * 128 : qt8 * 128 + 128, ph * PH : ph * PH + PH],
                        ostd_t[:],
                    )
                    nc.sync.dma_start(
                        mean_o[qt8 * 128 : qt8 * 128 + 128, ph * PH : ph * PH + PH],
                        omean_t[:],
                    )


def _prep_inputs(query_features, prototypes, weight_mu, weight_rho, bias_mu, bias_rho, eps_w, eps_b):
    f32, f16 = np.float32, np.float16
    sp_w = np.log1p(np.exp(weight_rho.astype(np.float64))).astype(f32)
    sp_b = np.log1p(np.exp(bias_rho.astype(np.float64))).astype(f32)
    W = (weight_mu[None] + eps_w * sp_w[None]).astype(f32)  # [S,D,D]
    B = (bias_mu[None] + eps_b * sp_b[None]).astype(f32)  # [S,D]
    Wh = W.astype(f16)
    qfh = query_features.astype(f16)  # [Q,D]

    yh = prototypes.astype(f16)  # [P,D]
    pn = (yh.astype(f32) ** 2).sum(-1, dtype=f32)  # [P]
    pn16q = pn.astype(f16)[None, :]  # [1,P] chain seed row
    pn10_16 = (float(S) * pn16q.astype(f32)).astype(f16)  # [1,P]
    b2 = (-2.0 * B).astype(f32)  # [S,D]

    W16 = np.ascontiguousarray(
        Wh.reshape(S, DT, 128, 256).transpose(0, 2, 1, 3).reshape(S, 128, DT * 256)
    )
    b2T = np.ascontiguousarray(
        b2.T.reshape(ET, 128, S).transpose(1, 0, 2).reshape(128, ET * S)
    )
    yT16 = np.ascontiguousarray(
        yh.T.reshape(ET, 128, P).transpose(1, 0, 2)
    )  # [128, ET, P]
    common = {
        "W16": W16,
        "b2T": b2T,
        "yT16": yT16,
        "pn16q": pn16q,
        "pn10_16": pn10_16,
        "onesr16": np.ones((1, 128), f16),
        "o16c": np.ones((128, 1), f16),
        "eye16": np.eye(128, dtype=f16),
    }
    in_maps = []
    for c in range(NCORES):
        qs = qfh[c * QLOC : (c + 1) * QLOC]  # [QLOC, D]
        qT16 = np.ascontiguousarray(
            qs.T.reshape(DT, 128, QLOC).transpose(1, 0, 2).reshape(128, DT * QLOC)
        )
        in_maps.append({"qT16": qT16, **common})
    return in_maps


def kernel(**inputs):
    global LAST_RESULTS
    n_samples = int(inputs.pop("n_samples", S))
    assert n_samples == S, f"kernel hardcodes S={S}, got {n_samples}"
    np_inputs = {
        k: np.asarray(v, dtype=np.float32)
        for k, v in inputs.items()
    }
    in_maps = _prep_inputs(**np_inputs)

    if "nc" not in _CACHE:
        _CACHE["nc"] = _build_bass()
    nc = _CACHE["nc"]

    trace = bool(int(os.environ.get("KERNEL_TRACE", "0")))
    res = bass_utils.run_bass_kernel_spmd(
        nc, in_maps, core_ids=list(range(NCORES)), trace=trace
    )
    LAST_RESULTS = res
    mean = np.concatenate([r["mean_o"] for r in res.results], axis=0)
    std = np.concatenate([r["std_o"] for r in res.results], axis=0)
    return mean, std


# revision 72
# speedup vs baseline: 1.5874x; 1.1220x over previous
"""Bayesian uncertainty distance kernel for TRN2 (8 NeuronCores, SPMD).

Math (per reference):
    W_s  = weight_mu + eps_w[s] * softplus(weight_rho)          [S,D,D]
    b_s  = bias_mu   + eps_b[s] * softplus(bias_rho)            [S,D]
    qt_s = query @ W_s + b_s                                    [S,Q,D]
    d2_s = ||qt_s||^2 - 2 qt_s.proto^T + ||proto||^2            [S,Q,P]
    mean = mean_s sqrt(d2_s);  std = std_s(sqrt(d2_s), ddof=1)

Sharding: data-parallel over Q (8192 -> 8 x 1024). Everything else replicated.

Design (per core, Q=1024, P=2048, D=256, S=10), ~342us measured:
  - samples are DEFINED as x_s := fp16(-2*(query@W_s + b_s)); every moment
    derives consistently from these values so rounding cancels in the
    variance to first order.
  - phase 1 per s: fp16 qt matmuls -> x_s = DVE tensor_scalar(psum*-2-2b);
    delta d_s = x_s - x_{s-2} (DVE fp16; drift ~2^-11|d| per step -- fp8
    deltas measured 2.3e-2 std error, fp16 keeps it at 6.7e-3);
    x2 = ACT Square(x_s); qn COLUMNS via width-1 PE matmuls (lhsT = x2
    128-col slices, rhs = ones col) into one [128, 80] psum tile -- they
    feed the phase-2 Sqrt as per-partition biases, which is what lets the
    rank-2 qn reseeding (25% of baseline PE issue slots) disappear;
    xsum psum += eye16 @ x_s.
  - phase 2 per qtile: FOUR [128,1024] psum chains (even/odd samples x
    lo/hi p-halves).  Each chain is seeded once with the rank-1 pn row,
    then per sample only the delta cross accumulates (K=2x128 fp16,
    start=False groups).  4 chains (vs 2 full-width) double the psum
    buffering so the per-chain PE->ACT WAR serialization overlaps; this
    was worth 88us of span.  dist = ACT Sqrt(chain + qn_s bias);
    macc += dist on DVE in exact fp32 (a PE float32r identity-matmul
    accumulation measured 1e-4 rel rounding which the variance amplifies
    ~360x into 0.16 std rms -- unusable).  fp8 DoubleRow crosses measured
    ~380ns/instr vs fp16's ~216ns on this stack (pessimization, reverted).
  - variance via sum-of-d2: ss = rank-1 ones x 10pn + xsum16 @ yT16;
    u = ss - (macc/S)^2*S (m2 on ACT Square); std = Sqrt(u/(S-1) + qn9
    bias) where qn9 = qnsum/(S-1) columns.  mean = macc/S (DVE).

The host does only O(S*D^2) prep in numpy (softplus, W_s, transposes, pn).
"""

import os
import numpy as np

import concourse.bass as bass
import concourse.mybir as mybir
import concourse.tile as tile
from concourse import bacc, bass_utils

AF = mybir.ActivationFunctionType
ALU = mybir.AluOpType

F32 = mybir.dt.float32
F16 = mybir.dt.float16

NCORES = 8
D = 256
Q_FULL = 8192
P = 2048
S = 10
QLOC = Q_FULL // NCORES  # 1024
ET = D // 128  # 2 e-tiles
DT = D // 128  # 2 d-tiles
QT = QLOC // 128  # 8 q-tiles per core
PH = 2048  # phase-2 psum tile width (4 banks)
NPH = P // PH  # 1



_CACHE = {}
LAST_RESULTS = None


def _build_bass():
    nc = bacc.Bacc(
        "TRN2",
        target_bir_lowering=False,
        debug=False,
        num_devices=NCORES,
    )
    ins = {}
    ins["qT16"] = nc.dram_tensor("qT16", [128, DT * QLOC], F16, kind="ExternalInput").ap()
    ins["W16"] = nc.dram_tensor("W16", [S, 128, DT * 256], F16, kind="ExternalInput").ap()
    ins["b2T"] = nc.dram_tensor("b2T", [128, ET * S], F32, kind="ExternalInput").ap()
    ins["yT16"] = nc.dram_tensor("yT16", [128, ET, P], F16, kind="ExternalInput").ap()
    ins["pn16q"] = nc.dram_tensor("pn16q", [1, P], F16, kind="ExternalInput").ap()
    ins["pn10_16"] = nc.dram_tensor("pn10_16", [1, P], F16, kind="ExternalInput").ap()
    ins["onesr16"] = nc.dram_tensor("onesr16", [1, 128], F16, kind="ExternalInput").ap()
    ins["o16c"] = nc.dram_tensor("o16c", [128, 1], F16, kind="ExternalInput").ap()
    ins["eye16"] = nc.dram_tensor("eye16", [128, 128], F16, kind="ExternalInput").ap()
    mean_o = nc.dram_tensor("mean_o", [QLOC, P], F32, kind="ExternalOutput").ap()
    std_o = nc.dram_tensor("std_o", [QLOC, P], F32, kind="ExternalOutput").ap()

    with tile.TileContext(nc) as tc:
        _kernel_body(tc, ins, mean_o, std_o)
    nc.compile()
    return nc


def _kernel_body(tc, ins, mean_o, std_o):
    nc = tc.nc
    from contextlib import ExitStack

    ctx = ExitStack()
    with ctx:
        cpool = ctx.enter_context(tc.tile_pool(name="consts", bufs=1))
        wpool = ctx.enter_context(tc.tile_pool(name="wpool", bufs=2))
        dpool = ctx.enter_context(tc.tile_pool(name="dpool", bufs=S))
        x16pool = ctx.enter_context(tc.tile_pool(name="x16p", bufs=3))
        x2pool = ctx.enter_context(tc.tile_pool(name="x2pool", bufs=2))
        xsumpool = ctx.enter_context(tc.tile_pool(name="xsumpool", bufs=1))
        qnpool = ctx.enter_context(tc.tile_pool(name="qnpool", bufs=1))
        distpool = ctx.enter_context(tc.tile_pool(name="distpool", bufs=4))
        finpool = ctx.enter_context(tc.tile_pool(name="finpool", bufs=2))
        outpool = ctx.enter_context(tc.tile_pool(name="outpool", bufs=3))

        # ---- constants into SBUF ----
        qT_t = cpool.tile([128, DT * QLOC], F16)
        nc.sync.dma_start(qT_t[:], ins["qT16"])
        b2_t = cpool.tile([128, ET * S], F32)
        nc.sync.dma_start(b2_t[:], ins["b2T"])
        yT16_t = cpool.tile([128, ET, P], F16)
        nc.sync.dma_start(yT16_t[:], ins["yT16"])
        pn16q_t = cpool.tile([1, P], F16)
        nc.sync.dma_start(pn16q_t[:], ins["pn16q"])
        pn10_t = cpool.tile([1, P], F16)
        nc.sync.dma_start(pn10_t[:], ins["pn10_16"])
        onesr16_t = cpool.tile([1, 128], F16)
        nc.sync.dma_start(onesr16_t[:], ins["onesr16"])
        o16c_t = cpool.tile([128, 1], F16)
        nc.sync.dma_start(o16c_t[:], ins["o16c"])
        eye16_t = cpool.tile([128, 128], F16)
        nc.sync.dma_start(eye16_t[:], ins["eye16"])

        xsum16_t = xsumpool.tile([128, ET, QLOC], F16)
        # qn columns: [128, QT, S] fp32; [128,1] slices feed the ACT Sqrt bias
        qncol_t = qnpool.tile([128, QT, S], F32)
        qn9r_t = qnpool.tile([128, QT], F32)
        qn9_t = qnpool.tile([128, QT], F32)  # qnsum/(S-1) bias columns for std

        x_tiles = []
        # ---------- phase 1: per-sample transformed queries + deltas ----------
        with tc.tile_pool(name="pp1", bufs=2, space="PSUM") as pp1, \
             tc.tile_pool(name="ppqn", bufs=1, space="PSUM") as ppqn, \
             tc.tile_pool(name="ppxs", bufs=1, space="PSUM") as ppxs:
            qncolp = ppqn.tile([128, QT * S], F32)
            xsump = ppxs.tile([128, ET * QLOC], F32)
            x16_list = []
            for s in range(S):
                w_t = wpool.tile([128, DT * 256], F16, tag="w")
                nc.sync.dma_start(w_t[:], ins["W16"][s])
                # s<2 tiles start the chains and must survive into
                # phase 2, so they come from the persistent delta pool
                if s < 2:
                    x16_s = dpool.tile([128, ET, QLOC], F16, tag="d16", name=f"x16_{s}")
                else:
                    x16_s = x16pool.tile([128, ET, QLOC], F16, tag="x16", name=f"x16_{s}")
                x16_list.append(x16_s)
                for et in range(ET):
                    for qc in range(2):
                        qp = pp1.tile([128, 512], F32, tag="ps", name=f"qp{s}_{et}_{qc}")
                        for dt_ in range(DT):
                            nc.tensor.matmul(
                                qp[:],
                                lhsT=w_t[:, dt_ * 256 + et * 128 : dt_ * 256 + et * 128 + 128],
                                rhs=qT_t[:, dt_ * QLOC + qc * 512 : dt_ * QLOC + qc * 512 + 512],
                                start=(dt_ == 0),
                                stop=(dt_ == DT - 1),
                            )
                        # x16 = fp16(-2*qt - 2*b) from psum on DVE
                        nc.vector.tensor_scalar(
                            x16_s[:, et, qc * 512 : qc * 512 + 512],
                            qp[:],
                            -2.0,
                            b2_t[:, et * S + s : et * S + s + 1],
                            ALU.mult,
                            ALU.add,
                        )
                # delta chains over stride-2 samples: the phase-2 psum keeps
                # pn + x_path.y alive across the chain, so only the delta is
                # multiplied each step (no per-sample rank-2 reseeding, which
                # was ~25% of all PE matmul issue slots).  fp16 deltas round
                # at ~2^-11|delta| per step, small enough for the std (fp8
                # deltas measured 2.3e-2 std error; fp16 keeps it at ~4e-3).
                if s < 2:
                    d16_s = x16_s
                else:
                    d16_s = dpool.tile([128, ET, QLOC], F16, tag="d16", name=f"dd{s}")
                    nc.vector.tensor_tensor(
                        d16_s[:], x16_s[:], x16_list[s - 2][:], ALU.subtract
                    )
                x_tiles.append(d16_s)
                x2_s = x2pool.tile([128, ET, QLOC], F16, tag="x2", name=f"x2_{s}")
                nc.scalar.square(x2_s[:], x16_s[:])
                # qn columns: width-1 matmuls, one column per (qtile, s)
                for qt8 in range(QT):
                    for et in range(ET):
                        nc.tensor.matmul(
                            qncolp[:, qt8 * S + s : qt8 * S + s + 1],
                            lhsT=x2_s[:, et, qt8 * 128 : qt8 * 128 + 128],
                            rhs=o16c_t[:],
                            start=(et == 0),
                            stop=(et == ET - 1),
                            skip_group_check=True,
                        )
                # xsum += x16_s (exact: eye16 matmuls, psum fp32)
                for et in range(ET):
                    for qc in range(2):
                        nc.tensor.matmul(
                            xsump[:, et * QLOC + qc * 512 : et * QLOC + qc * 512 + 512],
                            lhsT=eye16_t[:],
                            rhs=x16_s[:, et, qc * 512 : qc * 512 + 512],
                            start=(s == 0),
                            stop=(s == S - 1),
                            skip_group_check=True,
                        )
            # qn = 0.25 * sum x^2   (x = -2(qt+b))
            nc.vector.tensor_scalar_mul(
                qncol_t[:].rearrange("p a b -> p (a b)"), qncolp[:], 0.25
            )
            # qnsum/(S-1) columns for the std bias (qncol already has the 0.25)
            nc.vector.tensor_reduce(
                qn9r_t[:], qncol_t[:], axis=mybir.AxisListType.X, op=ALU.add
            )
            nc.vector.tensor_scalar_mul(qn9_t[:], qn9r_t[:], 1.0 / (S - 1))
            nc.vector.tensor_copy(
                xsum16_t[:].rearrange("p a b -> p (a b)"), xsump[:]
            )

        # ---------- phase 2: distances, moments, outputs ----------
        with tc.tile_pool(name="ppC", bufs=4, space="PSUM") as ppC, \
             tc.tile_pool(name="maccpool", bufs=2) as maccpool:
            for qt8 in range(QT):
                for ph in range(NPH):
                    macc_t = maccpool.tile([128, PH], F32, tag="macc", name=f"m{qt8}_{ph}")
                    # 4 half-width chains (A/B samples x lo/hi p-halves): same
                    # matmul+LDW count as 2 full-width chains but twice the
                    # independent psum buffers, so the per-chain PE->ACT->PE
                    # WAR serialization overlaps across halves
                    chains = [
                        ppC.tile([128, PH // 2], F32, tag="ps", name=f"ch{qt8}_{ph}_{ab}")
                        for ab in range(4)
                    ]
                    for ci, cp in enumerate(chains):
                        hb = (ci // 2) * (PH // 2)
                        for c in range(PH // 1024):
                            o = ph * PH + hb + c * 512
                            # pn seed: rank-1 ones x pn16q (fp16), once per chain
                            nc.tensor.matmul(
                                cp[:, c * 512 : c * 512 + 512],
                                lhsT=onesr16_t[:],
                                rhs=pn16q_t[:, o : o + 512],
                                start=True,
                                stop=True,
                                skip_group_check=True,
                            )
                    for s in range(S):
                        d16_s = x_tiles[s]
                        for et in range(ET):
                            lhs = d16_s[:, et, qt8 * 128 : qt8 * 128 + 128]
                            for hf in range(2):
                                cp = chains[s % 2 + 2 * hf]
                                for c in range(PH // 1024):
                                    o = ph * PH + hf * (PH // 2) + c * 512
                                    # delta cross accumulates onto the live chain
                                    nc.tensor.matmul(
                                        cp[:, c * 512 : c * 512 + 512],
                                        lhsT=lhs,
                                        rhs=yT16_t[:, et, o : o + 512],
                                        start=False,
                                        stop=(et == ET - 1),
                                        skip_group_check=True,
                                    )
                        # dist straight into macc for s=0, else via a rotating
                        # fp32 tile + exact DVE add (macc must be exact fp32:
                        # a PE f32r accumulation measured 1e-4 rel rounding,
                        # which the variance amplifies 360x -> std absmax 2.7)
                        dst = (
                            macc_t
                            if s == 0
                            else distpool.tile(
                                [128, PH], F32, tag="dist", name=f"d{qt8}_{ph}_{s}"
                            )
                        )
                        for hf in range(2):
                            nc.scalar.activation(
                                dst[:, hf * (PH // 2) : (hf + 1) * (PH // 2)],
                                chains[s % 2 + 2 * hf][:],
                                AF.Sqrt,
                                bias=qncol_t[:, qt8, s : s + 1],
                                scale=1.0,
                            )
                        if s > 0:
                            nc.vector.tensor_add(macc_t[:], macc_t[:], dst[:])
                    # ss = 10*pn + xsum.proto^T (fp16 cross, consistent)
                    ssps = [
                        ppC.tile([128, PH // 2], F32, tag="ps", name=f"ss{qt8}_{ph}_{hf}")
                        for hf in range(2)
                    ]
                    for hf in range(2):
                        for c in range(PH // 1024):
                            o = ph * PH + hf * (PH // 2) + c * 512
                            nc.tensor.matmul(
                                ssps[hf][:, c * 512 : c * 512 + 512],
                                lhsT=onesr16_t[:],
                                rhs=pn10_t[:, o : o + 512],
                                start=True,
                                stop=False,
                                skip_group_check=True,
                            )
                    for et in range(ET):
                        lhs = xsum16_t[:, et, qt8 * 128 : qt8 * 128 + 128]
                        for hf in range(2):
                            for c in range(PH // 1024):
                                o = ph * PH + hf * (PH // 2) + c * 512
                                nc.tensor.matmul(
                                    ssps[hf][:, c * 512 : c * 512 + 512],
                                    lhsT=lhs,
                                    rhs=yT16_t[:, et, o : o + 512],
                                    start=False,
                                    stop=(et == ET - 1),
                                    skip_group_check=True,
                                )
                    # drain ss to SBUF right away so its psum banks recycle
                    # into the next qtile's chains without waiting on the
                    # serial finals tail (macc -> m2 -> u)
                    ss_t = finpool.tile([128, PH], F32, tag="ss", name=f"ssb{qt8}_{ph}")
                    for hf in range(2):
                        sl = slice(hf * (PH // 2), (hf + 1) * (PH // 2))
                        nc.vector.tensor_copy(ss_t[:, sl], ssps[hf][:])
                    # omean = macc/S (DVE) and m2 = (macc/S)^2 (ACT Square
                    # with scale) both hang off macc directly and overlap
                    omean_t = outpool.tile([128, PH], F32, tag="out", name=f"om{qt8}_{ph}")
                    nc.vector.tensor_scalar_mul(omean_t[:], macc_t[:], 1.0 / S)
                    m2_t = finpool.tile([128, PH], F32, tag="fin", name=f"m2{qt8}_{ph}")
                    nc.scalar.activation(m2_t[:], macc_t[:], AF.Square, scale=1.0 / S)
                    u_t = finpool.tile([128, PH], F32, tag="fin", name=f"u{qt8}_{ph}")
                    nc.vector.scalar_tensor_tensor(
                        u_t[:], m2_t[:], -float(S), ss_t[:], ALU.mult, ALU.add
                    )
                    ostd_t = outpool.tile([128, PH], F32, tag="out", name=f"os{qt8}_{ph}")
                    nc.scalar.activation(
                        ostd_t[:], u_t[:], AF.Sqrt,
                        bias=qn9_t[:, qt8 : qt8 + 1],
                        scale=1.0 / (S - 1),
                    )
                    nc.sync.dma_start(
                        std_o[qt8 * 128 : qt8 * 128 + 128, ph * PH : ph * PH + PH],
                        ostd_t[:],
                    )
                    nc.sync.dma_start(
                        mean_o[qt8 * 128 : qt8 * 128 + 128, ph * PH : ph * PH + PH],
                        omean_t[:],
                    )


def _prep_inputs(query_features, prototypes, weight_mu, weight_rho, bias_mu, bias_rho, eps_w, eps_b):
    f32, f16 = np.float32, np.float16
    sp_w = np.log1p(np.exp(weight_rho.astype(np.float64))).astype(f32)
    sp_b = np.log1p(np.exp(bias_rho.astype(np.float64))).astype(f32)
    W = (weight_mu[None] + eps_w * sp_w[None]).astype(f32)  # [S,D,D]
    B = (bias_mu[None] + eps_b * sp_b[None]).astype(f32)  # [S,D]
    Wh = W.astype(f16)
    qfh = query_features.astype(f16)  # [Q,D]

    yh = prototypes.astype(f16)  # [P,D]
    pn = (yh.astype(f32) ** 2).sum(-1, dtype=f32)  # [P]
    pn16q = pn.astype(f16)[None, :]  # [1,P] chain seed row
    pn10_16 = (float(S) * pn16q.astype(f32)).astype(f16)  # [1,P]
    b2 = (-2.0 * B).astype(f32)  # [S,D]

    W16 = np.ascontiguousarray(
        Wh.reshape(S, DT, 128, 256).transpose(0, 2, 1, 3).reshape(S, 128, DT * 256)
    )
    b2T = np.ascontiguousarray(
        b2.T.reshape(ET, 128, S).transpose(1, 0, 2).reshape(128, ET * S)
    )
    yT16 = np.ascontiguousarray(
        yh.T.reshape(ET, 128, P).transpose(1, 0, 2)
    )  # [128, ET, P]
    common = {
        "W16": W16,
        "b2T": b2T,
        "yT16": yT16,
        "pn16q": pn16q,
        "pn10_16": pn10_16,
        "onesr16": np.ones((1, 128), f16),
        "o16c": np.ones((128, 1), f16),
        "eye16": np.eye(128, dtype=f16),
    }
    in_maps = []
    for c in range(NCORES):
        qs = qfh[c * QLOC : (c + 1) * QLOC]  # [QLOC, D]
        qT16 = np.ascontiguousarray(
            qs.T.reshape(DT, 128, QLOC).transpose(1, 0, 2).reshape(128, DT * QLOC)
        )
        in_maps.append({"qT16": qT16, **common})
    return in_maps


def kernel(**inputs):
    global LAST_RESULTS
    n_samples = int(inputs.pop("n_samples", S))
    assert n_samples == S, f"kernel hardcodes S={S}, got {n_samples}"
    np_inputs = {
        k: np.asarray(v, dtype=np.float32)
        for k, v in inputs.items()
    }
    in_maps = _prep_inputs(**np_inputs)

    if "nc" not in _CACHE:
        _CACHE["nc"] = _build_bass()
    nc = _CACHE["nc"]

    trace = bool(int(os.environ.get("KERNEL_TRACE", "0")))
    res = bass_utils.run_bass_kernel_spmd(
        nc, in_maps, core_ids=list(range(NCORES)), trace=trace
    )
    LAST_RESULTS = res
    mean = np.concatenate([r["mean_o"] for r in res.results], axis=0)
    std = np.concatenate([r["std_o"] for r in res.results], axis=0)
    return mean, std


# revision 75
# speedup vs baseline: 1.6014x; 1.0088x over previous
"""Bayesian uncertainty distance kernel for TRN2 (8 NeuronCores, SPMD).

Math (per reference):
    W_s  = weight_mu + eps_w[s] * softplus(weight_rho)          [S,D,D]
    b_s  = bias_mu   + eps_b[s] * softplus(bias_rho)            [S,D]
    qt_s = query @ W_s + b_s                                    [S,Q,D]
    d2_s = ||qt_s||^2 - 2 qt_s.proto^T + ||proto||^2            [S,Q,P]
    mean = mean_s sqrt(d2_s);  std = std_s(sqrt(d2_s), ddof=1)

Sharding: data-parallel over Q (8192 -> 8 x 1024). Everything else replicated.

Design (per core, Q=1024, P=2048, D=256, S=10), ~342us measured:
  - samples are DEFINED as x_s := fp16(-2*(query@W_s + b_s)); every moment
    derives consistently from these values so rounding cancels in the
    variance to first order.
  - phase 1 per s: fp16 qt matmuls -> x_s = DVE tensor_scalar(psum*-2-2b);
    delta d_s = x_s - x_{s-2} (DVE fp16; drift ~2^-11|d| per step -- fp8
    deltas measured 2.3e-2 std error, fp16 keeps it at 6.7e-3);
    x2 = ACT Square(x_s); qn COLUMNS via width-1 PE matmuls (lhsT = x2
    128-col slices, rhs = ones col) into one [128, 80] psum tile -- they
    feed the phase-2 Sqrt as per-partition biases, which is what lets the
    rank-2 qn reseeding (25% of baseline PE issue slots) disappear;
    xsum psum += eye16 @ x_s.
  - phase 2 per qtile: FOUR [128,1024] psum chains (even/odd samples x
    lo/hi p-halves).  Each chain is seeded once with the rank-1 pn row,
    then per sample only the delta cross accumulates (K=2x128 fp16,
    start=False groups).  4 chains (vs 2 full-width) double the psum
    buffering so the per-chain PE->ACT WAR serialization overlaps; this
    was worth 88us of span.  dist = ACT Sqrt(chain + qn_s bias);
    macc += dist on DVE in exact fp32 (a PE float32r identity-matmul
    accumulation measured 1e-4 rel rounding which the variance amplifies
    ~360x into 0.16 std rms -- unusable).  fp8 DoubleRow crosses measured
    ~380ns/instr vs fp16's ~216ns on this stack (pessimization, reverted).
  - variance via sum-of-d2: ss = rank-1 ones x 10pn + xsum16 @ yT16;
    u = ss - (macc/S)^2*S (m2 on ACT Square); std = Sqrt(u/(S-1) + qn9
    bias) where qn9 = qnsum/(S-1) columns.  mean = macc/S (DVE).

The host does only O(S*D^2) prep in numpy (softplus, W_s, transposes, pn).
"""

import os
import numpy as np

import concourse.bass as bass
import concourse.mybir as mybir
import concourse.tile as tile
from concourse import bacc, bass_utils

AF = mybir.ActivationFunctionType
ALU = mybir.AluOpType

F32 = mybir.dt.float32
F16 = mybir.dt.float16

NCORES = 8
D = 256
Q_FULL = 8192
P = 2048
S = 10
QLOC = Q_FULL // NCORES  # 1024
ET = D // 128  # 2 e-tiles
DT = D // 128  # 2 d-tiles
QT = QLOC // 128  # 8 q-tiles per core
PH = 2048  # phase-2 psum tile width (4 banks)
NPH = P // PH  # 1



_CACHE = {}
LAST_RESULTS = None


def _build_bass():
    nc = bacc.Bacc(
        "TRN2",
        target_bir_lowering=False,
        debug=False,
        num_devices=NCORES,
    )
    ins = {}
    ins["qT16"] = nc.dram_tensor("qT16", [128, DT * QLOC], F16, kind="ExternalInput").ap()
    ins["W16"] = nc.dram_tensor("W16", [S, 128, DT * 256], F16, kind="ExternalInput").ap()
    ins["b2T"] = nc.dram_tensor("b2T", [128, ET * S], F32, kind="ExternalInput").ap()
    ins["yT16"] = nc.dram_tensor("yT16", [128, ET, P], F16, kind="ExternalInput").ap()
    ins["pn16q"] = nc.dram_tensor("pn16q", [1, P], F16, kind="ExternalInput").ap()
    ins["pn10_16"] = nc.dram_tensor("pn10_16", [1, P], F16, kind="ExternalInput").ap()
    ins["onesr16"] = nc.dram_tensor("onesr16", [1, 128], F16, kind="ExternalInput").ap()
    ins["o16c"] = nc.dram_tensor("o16c", [128, 1], F16, kind="ExternalInput").ap()
    ins["eye16"] = nc.dram_tensor("eye16", [128, 128], F16, kind="ExternalInput").ap()
    mean_o = nc.dram_tensor("mean_o", [QLOC, P], F32, kind="ExternalOutput").ap()
    std_o = nc.dram_tensor("std_o", [QLOC, P], F32, kind="ExternalOutput").ap()

    with tile.TileContext(nc) as tc:
        _kernel_body(tc, ins, mean_o, std_o)
    nc.compile()
    return nc


def _kernel_body(tc, ins, mean_o, std_o):
    nc = tc.nc
    from contextlib import ExitStack

    ctx = ExitStack()
    with ctx:
        cpool = ctx.enter_context(tc.tile_pool(name="consts", bufs=1))
        wpool = ctx.enter_context(tc.tile_pool(name="wpool", bufs=2))
        dpool = ctx.enter_context(tc.tile_pool(name="dpool", bufs=S))
        x16pool = ctx.enter_context(tc.tile_pool(name="x16p", bufs=3))
        x2pool = ctx.enter_context(tc.tile_pool(name="x2pool", bufs=2))
        xsumpool = ctx.enter_context(tc.tile_pool(name="xsumpool", bufs=1))
        qnpool = ctx.enter_context(tc.tile_pool(name="qnpool", bufs=1))
        distpool = ctx.enter_context(tc.tile_pool(name="distpool", bufs=4))
        finpool = ctx.enter_context(tc.tile_pool(name="finpool", bufs=2))
        outpool = ctx.enter_context(tc.tile_pool(name="outpool", bufs=3))

        # ---- constants into SBUF ----
        qT_t = cpool.tile([128, DT * QLOC], F16)
        nc.sync.dma_start(qT_t[:], ins["qT16"])
        b2_t = cpool.tile([128, ET * S], F32)
        nc.sync.dma_start(b2_t[:], ins["b2T"])
        # yT16/pn are first used in phase 2; their DMAs are issued after the
        # first phase-1 weight loads so the pipeline starts sooner
        yT16_t = cpool.tile([128, ET, P], F16)
        pn16q_t = cpool.tile([1, P], F16)
        pn10_t = cpool.tile([1, P], F16)
        onesr16_t = cpool.tile([1, 128], F16)
        nc.sync.dma_start(onesr16_t[:], ins["onesr16"])
        o16c_t = cpool.tile([128, 1], F16)
        nc.sync.dma_start(o16c_t[:], ins["o16c"])
        eye16_t = cpool.tile([128, 128], F16)
        nc.sync.dma_start(eye16_t[:], ins["eye16"])

        xsum16_t = xsumpool.tile([128, ET, QLOC], F16)
        # qn columns: [128, QT, S] fp32; [128,1] slices feed the ACT Sqrt bias
        qncol_t = qnpool.tile([128, QT, S], F32)
        qn9r_t = qnpool.tile([128, QT], F32)
        qn9_t = qnpool.tile([128, QT], F32)  # qnsum/(S-1) bias columns for std

        x_tiles = []
        # ---------- phase 1: per-sample transformed queries + deltas ----------
        with tc.tile_pool(name="pp1", bufs=2, space="PSUM") as pp1, \
             tc.tile_pool(name="ppqn", bufs=1, space="PSUM") as ppqn, \
             tc.tile_pool(name="ppxs", bufs=1, space="PSUM") as ppxs:
            qncolp = ppqn.tile([128, QT * S], F32)
            xsump = ppxs.tile([128, ET * QLOC], F32)
            x16_list = []
            for s in range(S):
                w_t = wpool.tile([128, DT * 256], F16, tag="w")
                nc.sync.dma_start(w_t[:], ins["W16"][s])
                if s == 1:
                    nc.sync.dma_start(yT16_t[:], ins["yT16"])
                    nc.sync.dma_start(pn16q_t[:], ins["pn16q"])
                    nc.sync.dma_start(pn10_t[:], ins["pn10_16"])
                # s<2 tiles start the chains and must survive into
                # phase 2, so they come from the persistent delta pool
                if s < 2:
                    x16_s = dpool.tile([128, ET, QLOC], F16, tag="d16", name=f"x16_{s}")
                else:
                    x16_s = x16pool.tile([128, ET, QLOC], F16, tag="x16", name=f"x16_{s}")
                x16_list.append(x16_s)
                for et in range(ET):
                    for qc in range(2):
                        qp = pp1.tile([128, 512], F32, tag="ps", name=f"qp{s}_{et}_{qc}")
                        for dt_ in range(DT):
                            nc.tensor.matmul(
                                qp[:],
                                lhsT=w_t[:, dt_ * 256 + et * 128 : dt_ * 256 + et * 128 + 128],
                                rhs=qT_t[:, dt_ * QLOC + qc * 512 : dt_ * QLOC + qc * 512 + 512],
                                start=(dt_ == 0),
                                stop=(dt_ == DT - 1),
                            )
                        # x16 = fp16(-2*qt - 2*b) on ACT (Identity with the
                        # b2 bias column) -- DVE is the critical engine
                        nc.scalar.activation(
                            x16_s[:, et, qc * 512 : qc * 512 + 512],
                            qp[:],
                            AF.Identity,
                            bias=b2_t[:, et * S + s : et * S + s + 1],
                            scale=-2.0,
                        )
                # delta chains over stride-2 samples: the phase-2 psum keeps
                # pn + x_path.y alive across the chain, so only the delta is
                # multiplied each step (no per-sample rank-2 reseeding, which
                # was ~25% of all PE matmul issue slots).  fp16 deltas round
                # at ~2^-11|delta| per step, small enough for the std (fp8
                # deltas measured 2.3e-2 std error; fp16 keeps it at ~4e-3).
                if s < 2:
                    d16_s = x16_s
                else:
                    d16_s = dpool.tile([128, ET, QLOC], F16, tag="d16", name=f"dd{s}")
                    nc.vector.tensor_tensor(
                        d16_s[:], x16_s[:], x16_list[s - 2][:], ALU.subtract
                    )
                x_tiles.append(d16_s)
                x2_s = x2pool.tile([128, ET, QLOC], F16, tag="x2", name=f"x2_{s}")
                nc.scalar.square(x2_s[:], x16_s[:])
                # qn columns: width-1 matmuls, one column per (qtile, s)
                for qt8 in range(QT):
                    for et in range(ET):
                        nc.tensor.matmul(
                            qncolp[:, qt8 * S + s : qt8 * S + s + 1],
                            lhsT=x2_s[:, et, qt8 * 128 : qt8 * 128 + 128],
                            rhs=o16c_t[:],
                            start=(et == 0),
                            stop=(et == ET - 1),
                            skip_group_check=True,
                        )
                # xsum += x16_s (exact: eye16 matmuls, psum fp32)
                for et in range(ET):
                    for qc in range(2):
                        nc.tensor.matmul(
                            xsump[:, et * QLOC + qc * 512 : et * QLOC + qc * 512 + 512],
                            lhsT=eye16_t[:],
                            rhs=x16_s[:, et, qc * 512 : qc * 512 + 512],
                            start=(s == 0),
                            stop=(s == S - 1),
                            skip_group_check=True,
                        )
            # qn = 0.25 * sum x^2   (x = -2(qt+b))
            nc.vector.tensor_scalar_mul(
                qncol_t[:].rearrange("p a b -> p (a b)"), qncolp[:], 0.25
            )
            # qnsum/(S-1) columns for the std bias (qncol already has the 0.25)
            nc.vector.tensor_reduce(
                qn9r_t[:], qncol_t[:], axis=mybir.AxisListType.X, op=ALU.add
            )
            nc.vector.tensor_scalar_mul(qn9_t[:], qn9r_t[:], 1.0 / (S - 1))
            nc.vector.tensor_copy(
                xsum16_t[:].rearrange("p a b -> p (a b)"), xsump[:]
            )

        # ---------- phase 2: distances, moments, outputs ----------
        with tc.tile_pool(name="ppC", bufs=4, space="PSUM") as ppC, \
             tc.tile_pool(name="maccpool", bufs=2) as maccpool:
            for qt8 in range(QT):
                for ph in range(NPH):
                    macc_t = maccpool.tile([128, PH], F32, tag="macc", name=f"m{qt8}_{ph}")
                    # 4 half-width chains (A/B samples x lo/hi p-halves): same
                    # matmul+LDW count as 2 full-width chains but twice the
                    # independent psum buffers, so the per-chain PE->ACT->PE
                    # WAR serialization overlaps across halves
                    chains = [
                        ppC.tile([128, PH // 2], F32, tag="ps", name=f"ch{qt8}_{ph}_{ab}")
                        for ab in range(4)
                    ]
                    for ci, cp in enumerate(chains):
                        hb = (ci // 2) * (PH // 2)
                        for c in range(PH // 1024):
                            o = ph * PH + hb + c * 512
                            # pn seed: rank-1 ones x pn16q (fp16), once per chain
                            nc.tensor.matmul(
                                cp[:, c * 512 : c * 512 + 512],
                                lhsT=onesr16_t[:],
                                rhs=pn16q_t[:, o : o + 512],
                                start=True,
                                stop=True,
                                skip_group_check=True,
                            )
                    for s in range(S):
                        d16_s = x_tiles[s]
                        for et in range(ET):
                            lhs = d16_s[:, et, qt8 * 128 : qt8 * 128 + 128]
                            for hf in range(2):
                                cp = chains[s % 2 + 2 * hf]
                                for c in range(PH // 1024):
                                    o = ph * PH + hf * (PH // 2) + c * 512
                                    # delta cross accumulates onto the live chain
                                    nc.tensor.matmul(
                                        cp[:, c * 512 : c * 512 + 512],
                                        lhsT=lhs,
                                        rhs=yT16_t[:, et, o : o + 512],
                                        start=False,
                                        stop=(et == ET - 1),
                                        skip_group_check=True,
                                    )
                        # dist straight into macc for s=0, else via a rotating
                        # fp32 tile + exact DVE add (macc must be exact fp32:
                        # a PE f32r accumulation measured 1e-4 rel rounding,
                        # which the variance amplifies 360x -> std absmax 2.7)
                        dst = (
                            macc_t
                            if s == 0
                            else distpool.tile(
                                [128, PH], F32, tag="dist", name=f"d{qt8}_{ph}_{s}"
                            )
                        )
                        for hf in range(2):
                            nc.scalar.activation(
                                dst[:, hf * (PH // 2) : (hf + 1) * (PH // 2)],
                                chains[s % 2 + 2 * hf][:],
                                AF.Sqrt,
                                bias=qncol_t[:, qt8, s : s + 1],
                                scale=1.0,
                            )
                        if s > 0:
                            nc.vector.tensor_add(macc_t[:], macc_t[:], dst[:])
                    # ss = 10*pn + xsum.proto^T (fp16 cross, consistent)
                    ssps = [
                        ppC.tile([128, PH // 2], F32, tag="ps", name=f"ss{qt8}_{ph}_{hf}")
                        for hf in range(2)
                    ]
                    for hf in range(2):
                        for c in range(PH // 1024):
                            o = ph * PH + hf * (PH // 2) + c * 512
                            nc.tensor.matmul(
                                ssps[hf][:, c * 512 : c * 512 + 512],
                                lhsT=onesr16_t[:],
                                rhs=pn10_t[:, o : o + 512],
                                start=True,
                                stop=False,
                                skip_group_check=True,
                            )
                    for et in range(ET):
                        lhs = xsum16_t[:, et, qt8 * 128 : qt8 * 128 + 128]
                        for hf in range(2):
                            for c in range(PH // 1024):
                                o = ph * PH + hf * (PH // 2) + c * 512
                                nc.tensor.matmul(
                                    ssps[hf][:, c * 512 : c * 512 + 512],
                                    lhsT=lhs,
                                    rhs=yT16_t[:, et, o : o + 512],
                                    start=False,
                                    stop=(et == ET - 1),
                                    skip_group_check=True,
                                )
                    # drain ss to SBUF right away so its psum banks recycle
                    # into the next qtile's chains without waiting on the
                    # serial finals tail (macc -> m2 -> u)
                    ss_t = finpool.tile([128, PH], F32, tag="ss", name=f"ssb{qt8}_{ph}")
                    for hf in range(2):
                        sl = slice(hf * (PH // 2), (hf + 1) * (PH // 2))
                        nc.vector.tensor_copy(ss_t[:, sl], ssps[hf][:])
                    # omean = macc/S (DVE) and m2 = (macc/S)^2 (ACT Square
                    # with scale) both hang off macc directly and overlap
                    omean_t = outpool.tile([128, PH], F32, tag="out", name=f"om{qt8}_{ph}")
                    nc.vector.tensor_scalar_mul(omean_t[:], macc_t[:], 1.0 / S)
                    m2_t = finpool.tile([128, PH], F32, tag="fin", name=f"m2{qt8}_{ph}")
                    nc.scalar.activation(m2_t[:], macc_t[:], AF.Square, scale=1.0 / S)
                    u_t = finpool.tile([128, PH], F32, tag="fin", name=f"u{qt8}_{ph}")
                    nc.vector.scalar_tensor_tensor(
                        u_t[:], m2_t[:], -float(S), ss_t[:], ALU.mult, ALU.add
                    )
                    ostd_t = outpool.tile([128, PH], F32, tag="out", name=f"os{qt8}_{ph}")
                    nc.scalar.activation(
                        ostd_t[:], u_t[:], AF.Sqrt,
                        bias=qn9_t[:, qt8 : qt8 + 1],
                        scale=1.0 / (S - 1),
                    )
                    nc.sync.dma_start(
                        std_o[qt8 * 128 : qt8 * 128 + 128, ph * PH : ph * PH + PH],
                        ostd_t[:],
                    )
                    nc.sync.dma_start(
                        mean_o[qt8 * 128 : qt8 * 128 + 128, ph * PH : ph * PH + PH],
                        omean_t[:],
                    )


def _prep_inputs(query_features, prototypes, weight_mu, weight_rho, bias_mu, bias_rho, eps_w, eps_b):
    f32, f16 = np.float32, np.float16
    sp_w = np.log1p(np.exp(weight_rho.astype(np.float64))).astype(f32)
    sp_b = np.log1p(np.exp(bias_rho.astype(np.float64))).astype(f32)
    W = (weight_mu[None] + eps_w * sp_w[None]).astype(f32)  # [S,D,D]
    B = (bias_mu[None] + eps_b * sp_b[None]).astype(f32)  # [S,D]
    Wh = W.astype(f16)
    qfh = query_features.astype(f16)  # [Q,D]

    yh = prototypes.astype(f16)  # [P,D]
    pn = (yh.astype(f32) ** 2).sum(-1, dtype=f32)  # [P]
    pn16q = pn.astype(f16)[None, :]  # [1,P] chain seed row
    pn10_16 = (float(S) * pn16q.astype(f32)).astype(f16)  # [1,P]
    b2 = (-2.0 * B).astype(f32)  # [S,D]

    W16 = np.ascontiguousarray(
        Wh.reshape(S, DT, 128, 256).transpose(0, 2, 1, 3).reshape(S, 128, DT * 256)
    )
    b2T = np.ascontiguousarray(
        b2.T.reshape(ET, 128, S).transpose(1, 0, 2).reshape(128, ET * S)
    )
    yT16 = np.ascontiguousarray(
        yh.T.reshape(ET, 128, P).transpose(1, 0, 2)
    )  # [128, ET, P]
    common = {
        "W16": W16,
        "b2T": b2T,
        "yT16": yT16,
        "pn16q": pn16q,
        "pn10_16": pn10_16,
        "onesr16": np.ones((1, 128), f16),
        "o16c": np.ones((128, 1), f16),
        "eye16": np.eye(128, dtype=f16),
    }
    in_maps = []
    for c in range(NCORES):
        qs = qfh[c * QLOC : (c + 1) * QLOC]  # [QLOC, D]
        qT16 = np.ascontiguousarray(
            qs.T.reshape(DT, 128, QLOC).transpose(1, 0, 2).reshape(128, DT * QLOC)
        )
        in_maps.append({"qT16": qT16, **common})
    return in_maps


def kernel(**inputs):
    global LAST_RESULTS
    n_samples = int(inputs.pop("n_samples", S))
    assert n_samples == S, f"kernel hardcodes S={S}, got {n_samples}"
    np_inputs = {
        k: np.asarray(v, dtype=np.float32)
        for k, v in inputs.items()
    }
    in_maps = _prep_inputs(**np_inputs)

    if "nc" not in _CACHE:
        _CACHE["nc"] = _build_bass()
    nc = _CACHE["nc"]

    trace = bool(int(os.environ.get("KERNEL_TRACE", "0")))
    res = bass_utils.run_bass_kernel_spmd(
        nc, in_maps, core_ids=list(range(NCORES)), trace=trace
    )
    LAST_RESULTS = res
    mean = np.concatenate([r["mean_o"] for r in res.results], axis=0)
    std = np.concatenate([r["std_o"] for r in res.results], axis=0)
    return mean, std
